# revision 1
# baseline (speedup 1.0000x reference)
"""Trainium2 Bass kernel for the 6-node GCN classification model.

Math: each GCN layer is h' = relu(A @ h @ W^T + b) on [B, 6, 64], where A is
the 6x6 normalized adjacency (with self loops; fill=1.0 for layers 1-2,
fill=2.0 for layers 3-4).  With the 6 nodes stacked in pairs on the 128 SBUF
partitions (2 nodes x 64 features), the fused per-layer operator A (x) W is a
384x384 block matrix; each nonzero 128x128 block becomes one TensorE matmul
accumulated in PSUM.  Node pairings alternate between two stackings chosen so
the total block count over the 4 layers is the provable minimum (26).

Pipeline per 512-batch group, fully fused in SBUF (x is read from HBM exactly
once, only the [B, 6] sigmoid output is written back):
  DMA x (batch-major, contiguous) -> PE transpose to feature-major stacked
  -> 4 x (block matmuls f32r -> ACT bias+ReLU) -> DVE residual add
  -> fc-head matmuls -> ACT sigmoid -> PE transpose back -> DMA out.

Sharding: pure data parallel over the batch dim across the 8 NeuronCores.
"""

import math
from contextlib import ExitStack, nullcontext as _nullctx

import numpy as np

N_CORES = 8
BATCH = 131072
PER_CORE = BATCH // N_CORES  # 16384
NN = 6
FEAT = 64
GROUP = 512
N_GROUPS = PER_CORE // GROUP  # 32

SRC = [1, 2, 0, 2, 1, 3, 2, 4, 3, 5, 3, 4]
DST = [0, 0, 1, 1, 2, 2, 3, 3, 4, 4, 5, 5]

# Node pair stackings per layer boundary (chain start == chain end so the
# residual/fc read the same stacking the input transposes produce).
S_A = [(0, 1), (2, 3), (4, 5)]
S_B = [(0, 5), (1, 2), (3, 4)]
CHAIN = [S_A, S_B, S_A, S_B, S_A]  # layer l maps CHAIN[l] -> CHAIN[l+1]
_IPERMS = [(0, 1, 2), (0, 2, 1), (1, 0, 2), (1, 2, 0), (2, 0, 1), (2, 1, 0)]

# Engine assignments for the post-matmul elementwise ops.  GPSIMD (Pool)
# cannot access PSUM on TRN2, so the PSUM->SBUF relu moves and xt copies can
# only go to ACT (A) / DVE (V); Pool (P) can take SBUF->SBUF residual adds.
ENG_TABLES = {
    # baseline: ACT does everything except one relu per early layer on DVE
    "base": dict(
        relu=[("A", "A", "V"), ("A", "A", "V"), ("A", "A", "V"),
              ("A", "A", "A")],
        cp=("V", "V", "V"), add=("V", "V", "V")),
    # even ACT/DVE split, residual adds on Pool
    "split": dict(
        relu=[("A", "V", "A"), ("V", "A", "V"), ("A", "V", "A"),
              ("V", "A", "V")],
        cp=("V", "A", "V"), add=("P", "P", "P")),
    # even ACT/DVE split, residual adds on DVE
    "splitv": dict(
        relu=[("A", "V", "A"), ("V", "A", "V"), ("A", "V", "A"),
              ("V", "A", "V")],
        cp=("V", "A", "V"), add=("V", "V", "V")),
    # fp8+dmat balance: ACT carries 5 relus + sigmoid, DVE 7 relus, Pool adds
    "split8": dict(
        relu=[("A", "V", "V"), ("A", "V", "A"), ("V", "A", "V"),
              ("A", "V", "V")],
        cp=("V", "A", "V"), add=("P", "P", "P")),
}


def _gcn_A(fill: float) -> np.ndarray:
    """Dense [6, 6] aggregation matrix A[dst, src] incl. weighted self loops."""
    src = SRC + list(range(NN))
    dst = DST + list(range(NN))
    w = [1.0] * len(SRC) + [fill] * NN
    deg = np.zeros(NN, np.float64)
    for s, d, ww in zip(src, dst, w):
        deg[d] += ww
    dinv = np.where(deg > 0, 1.0 / np.sqrt(deg), 0.0)
    A = np.zeros((NN, NN), np.float64)
    for s, d, ww in zip(src, dst, w):
        A[d, s] += dinv[s] * ww * dinv[d]
    return A


def _block_plan():
    """Static plan: for each layer, the nonzero (out_tile, in_tile) blocks.

    Returns [layer][out_tile] -> list of in_tile indices, using the support of
    A (same for both fill values)."""
    S = np.zeros((NN, NN), bool)
    for s, d in zip(SRC, DST):
        S[d, s] = True
    for i in range(NN):
        S[i, i] = True
    plan = []
    for layer in range(4):
        inp, outp = CHAIN[layer], CHAIN[layer + 1]
        lplan = []
        for (n0, n1) in outp:
            js = []
            for j, (m0, m1) in enumerate(inp):
                if S[n0, m0] or S[n0, m1] or S[n1, m0] or S[n1, m1]:
                    js.append(j)
            lplan.append(js)
        plan.append(lplan)
    return plan


BLOCK_PLAN = _block_plan()
N_BLOCKS = sum(len(js) for lp in BLOCK_PLAN for js in lp)  # 26


def _pair_plan(fp8set):
    """DoubleRow pair schedule for the fp8 layers.

    Each entry: (layer, out_tile, rhs_j0, (blkA|None, blkB|None)) meaning one
    DoubleRow matmul with rhs = h[j0], h[j0+1] and the two stationary K-tiles
    holding A-scaled weight blocks (None = zero block).  blkX indexes into
    the flat wblk ordering used by build_consts."""
    woff = [0]
    for layer in range(4):
        for i in range(3):
            woff.append(woff[-1] + len(BLOCK_PLAN[layer][i]))
    plan = []
    for layer in sorted(fp8set):
        for i in range(3):
            js = BLOCK_PLAN[layer][i]
            k0 = woff[layer * 3 + i]
            ks = {j: k0 + bi for bi, j in enumerate(js)}
            if js == [0, 1] or js == [1, 2]:
                plan.append((layer, i, js[0], (ks[js[0]], ks[js[1]])))
            elif js == [0, 1, 2]:
                plan.append((layer, i, 0, (ks[0], ks[1])))
                plan.append((layer, i, 1, (None, ks[2])))
            elif js == [0, 2]:
                plan.append((layer, i, 0, (ks[0], None)))
                plan.append((layer, i, 1, (None, ks[2])))
            else:
                raise AssertionError(js)
    return plan


def build_consts(W, b, fc_w, fc_b):
    """Host-side constant tensors fed to the device as DRAM inputs.

    W: list of 4 [64, 64] arrays; b: list of 4 [64]; fc_w [6, 64]; fc_b [6].
    """
    A = [_gcn_A(1.0), _gcn_A(1.0), _gcn_A(2.0), _gcn_A(2.0)]
    wblk = np.zeros((N_BLOCKS, 128, 128), np.float32)
    k = 0
    for layer in range(4):
        inp, outp = CHAIN[layer], CHAIN[layer + 1]
        Wt = W[layer].T.astype(np.float64)  # [f, g] = W[g, f]
        for i, (n0, n1) in enumerate(outp):
            for j in BLOCK_PLAN[layer][i]:
                m0, m1 = inp[j]
                blk = np.zeros((128, 128), np.float64)
                for dj, m in enumerate((m0, m1)):
                    for do, n in enumerate((n0, n1)):
                        a = A[layer][n, m]
                        if a != 0.0:
                            blk[dj * 64:(dj + 1) * 64, do * 64:(do + 1) * 64] = a * Wt
                wblk[k] = blk.astype(np.float32)
                k += 1
    assert k == N_BLOCKS

    bias = np.zeros((4, 128), np.float32)
    for layer in range(4):
        bias[layer] = np.tile(b[layer], 2)

    fcw = np.zeros((3, 128, NN), np.float32)
    for i, (n0, n1) in enumerate(CHAIN[4]):
        for do, n in enumerate((n0, n1)):
            fcw[i, do * 64:(do + 1) * 64, n] = fc_w[n]

    return {
        "wblk": wblk,
        "bias": bias,
        "fcw": fcw,
        "fcb": fc_b.astype(np.float32).reshape(NN, 1),
        "eye128": np.eye(128, dtype=np.float32),
    }


def build_program(repeats: int = 1, cfg: dict | None = None):
    """Build + schedule + compile the Bass/Tile program. Returns nc."""
    import concourse.tile as tile
    import concourse.mybir as mybir
    from concourse import bacc

    cfg = dict(cfg or {})
    bufs_xb = cfg.get("xb", 3)
    bufs_xs = cfg.get("xs", 3)
    bufs_h = cfg.get("h", 4)
    bufs_r = cfg.get("r", 2)
    bufs_px = cfg.get("px", 2)
    bufs_ph = cfg.get("ph", 5)
    bufs_pfc = cfg.get("pfc", 1)
    bufs_pot = cfg.get("pot", 1)
    xt_in_ph = cfg.get("xt_in_ph", False)
    ot_in_pfc = cfg.get("ot_in_pfc", False)
    xdt_name = cfg.get("xdtype", "f32r")
    wdt_name = cfg.get("wdtype", xdt_name)
    dmat = cfg.get("dmat", False)  # XBAR DMA-transpose input path (bf16 only)
    hostt = cfg.get("hostt", False)  # host pre-transposed x upload
    if dmat or hostt:
        assert xdt_name == "bf16"
        bufs_ph = cfg.get("ph", 7)  # px pool unused -> spend its PSUM banks
    if cfg.get("bigps"):
        bufs_ph = cfg.get("ph", 2)  # [128, 3*GROUP] tiles = 3 banks each
    fp8set = FP8_SETS[cfg.get("fp8")]
    pairs = _pair_plan(fp8set) if fp8set else []
    ENG = ENG_TABLES[cfg.get("eng", "base")]
    RELU_ENG, CP_ENG, ADD_ENG = ENG["relu"], ENG["cp"], ENG["add"]

    f32 = mybir.dt.float32
    f32r = mybir.dt.float32r
    _DT = {"f32r": mybir.dt.float32r, "bf16": mybir.dt.bfloat16}
    wdt = _DT[wdt_name]
    adt = _DT[xdt_name]  # on-chip dtype of x and of the layer activations
    Relu = mybir.ActivationFunctionType.Relu
    Sigmoid = mybir.ActivationFunctionType.Sigmoid
    Copy = mybir.ActivationFunctionType.Copy
    f8 = mybir.dt.float8e4
    DR = mybir.MatmulPerfMode.DoubleRow

    nc = bacc.Bacc("TRN2", target_bir_lowering=False, debug=False,
                   num_devices=N_CORES)

    if hostt:
        xt_ap = nc.dram_tensor("xt", [N_GROUPS, 128, 3 * GROUP], adt,
                               kind="ExternalInput").ap()
        if 0 in fp8set:
            xt8_ap = nc.dram_tensor("xt8", [N_GROUPS, 128, 3 * GROUP],
                                    mybir.dt.float8e4,
                                    kind="ExternalInput").ap()
    else:
        x_ap = nc.dram_tensor("x", [PER_CORE, NN * FEAT], adt,
                              kind="ExternalInput").ap()
    y_ap = nc.dram_tensor("y", [N_GROUPS, NN, GROUP], f32,
                          kind="ExternalOutput").ap()
    wblk_ap = nc.dram_tensor("wblk", [N_BLOCKS, 128, 128], wdt,
                             kind="ExternalInput").ap()
    bias_ap = nc.dram_tensor("bias", [4, 128], f32,
                             kind="ExternalInput").ap()
    fcw_ap = nc.dram_tensor("fcw", [3, 128, NN], wdt,
                            kind="ExternalInput").ap()
    fcb_ap = nc.dram_tensor("fcb", [NN, 1], f32, kind="ExternalInput").ap()
    eye128_ap = nc.dram_tensor("eye128", [128, 128], adt,
                               kind="ExternalInput").ap()
    if pairs:
        wp8_ap = nc.dram_tensor("wpair8", [len(pairs), 128, 256], f8,
                                kind="ExternalInput").ap()

    SB = GROUP // 128  # 4 batch sub-tiles per group

    with tile.TileContext(nc) as tc, ExitStack() as ctx:
        cpool = ctx.enter_context(tc.tile_pool(name="consts", bufs=1))
        p_xb = ctx.enter_context(tc.tile_pool(name="xb", bufs=bufs_xb))
        p_xs = ctx.enter_context(tc.tile_pool(name="xs", bufs=bufs_xs))
        p_h = ctx.enter_context(tc.tile_pool(name="h", bufs=bufs_h))
        p_r = ctx.enter_context(tc.tile_pool(name="r", bufs=bufs_r))
        p_sig = ctx.enter_context(tc.tile_pool(name="sig", bufs=2))
        p_ob = ctx.enter_context(tc.tile_pool(name="ob", bufs=2))
        p_ph = ctx.enter_context(tc.tile_pool(name="ph", bufs=bufs_ph, space="PSUM"))
        p_px = p_ph if xt_in_ph else ctx.enter_context(
            tc.tile_pool(name="px", bufs=bufs_px, space="PSUM"))
        p_pot = ctx.enter_context(
            tc.tile_pool(name="pot", bufs=bufs_pot, space="PSUM"))
        eye128 = cpool.tile([128, 128], adt, tag="eye128")
        nc.sync.dma_start(eye128[:], eye128_ap[:])
        btile = cpool.tile([128, 4], f32, tag="bias")
        nc.sync.dma_start(btile[:], bias_ap.rearrange("l p -> p l"))
        bt = [btile[:, layer:layer + 1] for layer in range(4)]
        ftile = cpool.tile([128, 3 * NN], wdt, tag="fcw")
        nc.sync.dma_start(ftile[:].rearrange("p (i n) -> p i n", i=3),
                          fcw_ap.rearrange("i p n -> p i n"))
        fct = [ftile[:, i * NN:(i + 1) * NN] for i in range(3)]
        fcbt = cpool.tile([NN, 1], f32, tag="fcb")
        nc.sync.dma_start(fcbt[:], fcb_ap[:])
        def load_xb(g):
            xb = p_xb.tile([128, SB * NN * FEAT], adt, tag="xb")
            nc.sync.dma_start(
                xb[:].rearrange("p (s f) -> p s f", s=SB),
                x_ap[g * GROUP:(g + 1) * GROUP, :].rearrange(
                    "(s p) f -> p s f", p=128),
            )
            return xb

        def load_xs_t(g):
            # XBAR DMA transpose: [512, 384] bf16 rows -> feature-major
            # [128, 3, 512] (chunk j holds features 128j..128j+127, which is
            # exactly the S_A node-pair stacking).  Replaces 12 PE transposes
            # and 3 PSUM->SBUF copies per group.
            xst = p_xs.tile([128, 3 * GROUP], adt, tag="xs")
            nc.sync.dma_start_transpose(
                xst[:].rearrange("p (j b) -> p j b", j=3),
                x_ap[g * GROUP:(g + 1) * GROUP, :])
            return xst

        def load_xs_host(g):
            # Host pre-transposed x: plain contiguous DMA per group (bf16 for
            # layer-1/residual, plus fp8 copy when layer 1 runs DoubleRow).
            xst = p_xs.tile([128, 3 * GROUP], adt, tag="xs")
            nc.sync.dma_start(xst[:], xt_ap[g])
            x8 = None
            if 0 in fp8set:
                x8 = p_xs.tile([128, 3 * GROUP], f8, tag="xs8")
                nc.sync.dma_start(x8[:], xt8_ap[g])
            return (xst, x8)

        # The first groups' x tiles go before the big weight DMAs so the
        # transposes can start while the weights stream in.  (Single-pass
        # builds only: under a repeat loop these DMAs would not replay.)
        load_x_pre = (load_xs_host if hostt
                      else (load_xs_t if dmat else load_xb))
        xb_pre = ({g: load_x_pre(g) for g in range(min(2, N_GROUPS))}
                  if repeats == 1 else {})

        # Block weights in two DMAs (layer-0 blocks first so the first
        # group's matmuls can start before the rest of the weights land).
        nb0 = sum(len(js) for js in BLOCK_PLAN[0])
        wtile = cpool.tile([128, N_BLOCKS * 128], wdt, tag="wblk")
        nc.sync.dma_start(
            wtile[:, :nb0 * 128].rearrange("p (k f) -> p k f", k=nb0),
            wblk_ap[:nb0].rearrange("k p f -> p k f"))
        nc.sync.dma_start(
            wtile[:, nb0 * 128:].rearrange("p (k f) -> p k f", k=N_BLOCKS - nb0),
            wblk_ap[nb0:].rearrange("k p f -> p k f"))
        wt = [wtile[:, k * 128:(k + 1) * 128] for k in range(N_BLOCKS)]

        if pairs:
            wp8 = cpool.tile([128, len(pairs) * 256], f8, tag="wpair8")
            nc.sync.dma_start(
                wp8[:].rearrange("p (k f) -> p k f", k=len(pairs)),
                wp8_ap.rearrange("k p f -> p k f"))
            pair_by_li = {}
            for pi, (pl, i, j0, _ks) in enumerate(pairs):
                pair_by_li.setdefault((pl, i), []).append((pi, j0))


        probe = cfg.get("probe", "")

        def relu_move(e, ht, ps, layer):
            if "tinyrelu" in probe:
                # Probe mode: keep the dataflow edges but shrink the
                # elementwise work to ~nothing (timing only, wrong numerics).
                nc.scalar.activation(ht[:, :16], ps[:, :16], Relu,
                                     bias=bt[layer])
                return
            if cfg.get("relu2"):
                # Halve the relu latency on the critical path: ACT and DVE
                # each process half the tile concurrently.
                half = GROUP // 2
                nc.scalar.activation(ht[:, :half], ps[:, :half], Relu,
                                     bias=bt[layer])
                nc.vector.tensor_scalar(out=ht[:, half:], in0=ps[:, half:],
                                        scalar1=bt[layer], scalar2=0.0,
                                        op0=mybir.AluOpType.add,
                                        op1=mybir.AluOpType.max)
                return
            if e == "A":
                nc.scalar.activation(ht[:], ps[:], Relu, bias=bt[layer])
            else:
                eng = nc.vector if e == "V" else nc.gpsimd
                eng.tensor_scalar(out=ht[:], in0=ps[:], scalar1=bt[layer],
                                  scalar2=0.0, op0=mybir.AluOpType.add,
                                  op1=mybir.AluOpType.max)

        def group_body(g):
            first_t = None
            xs8t_loaded = None
            if hostt:
                pre = xb_pre.pop(g, None)
                xst, xs8t_loaded = pre if pre is not None else load_xs_host(g)
                xs = [xst[:, j * GROUP:(j + 1) * GROUP] for j in range(3)]
            elif dmat:
                xst = xb_pre.pop(g, None)
                if xst is None:
                    xst = load_xs_t(g)
                xs = [xst[:, j * GROUP:(j + 1) * GROUP] for j in range(3)]
            else:
                # Load [512, 384] rows batch-major: partition = batch % 128.
                xb = xb_pre.pop(g, None)
                if xb is None:
                    xb = load_xb(g)
                # Transpose to feature-major stacked (pairs = CHAIN[0]).
                xs = []
                xts = []
                for j in range(3):
                    xt = p_px.tile([128, GROUP], adt,
                                   tag="ph" if xt_in_ph else "xt")
                    for s in range(SB):
                        ti = nc.tensor.transpose(
                            xt[:, s * 128:(s + 1) * 128],
                            xb[:, s * NN * FEAT + j * 128:
                               s * NN * FEAT + (j + 1) * 128],
                            eye128[:],
                        )
                        if first_t is None:
                            first_t = ti
                    xts.append(xt)
                for j in range(3):
                    xsj = p_xs.tile([128, GROUP], adt, tag=f"xs{j}")
                    e = CP_ENG[j]
                    if e == "A":
                        nc.scalar.activation(xsj[:], xts[j][:], Copy)
                    else:
                        eng = nc.vector if e == "V" else nc.gpsimd
                        eng.tensor_copy(out=xsj[:], in_=xts[j][:])
                    xs.append(xsj)

            h = xs
            iperm = _IPERMS[cfg.get("iorder", 0)]
            woff = [0]
            for layer in range(4):
                for i in range(3):
                    woff.append(woff[-1] + len(BLOCK_PLAN[layer][i]))

            # h3: [p, t, b] single-tile view of the CURRENT h list, required
            # as the DoubleRow rhs of an fp8 layer.
            if 0 in fp8set:
                if xs8t_loaded is not None:
                    h3 = xs8t_loaded[:].rearrange("p (t b) -> p t b", t=3)
                else:
                    xs8t = p_xs.tile([128, 3 * GROUP], f8, tag="xs8")
                    for j in range(3):
                        if CP_ENG[j] == "A":
                            nc.scalar.activation(
                                xs8t[:, j * GROUP:(j + 1) * GROUP],
                                xs[j][:], Copy)
                        else:
                            nc.vector.tensor_copy(
                                out=xs8t[:, j * GROUP:(j + 1) * GROUP],
                                in_=xs[j][:])
                    h3 = xs8t[:].rearrange("p (t b) -> p t b", t=3)
            else:
                h3 = None

            bigps = cfg.get("bigps", False)

            def new_h(layer):
                """Output container for a layer: (slices, [p,t,b] view or
                None, full [128,1536] AP or None).  Single tile when the next
                layer consumes it via DoubleRow or when bigps needs span
                writes."""
                if (layer + 1) in fp8set:
                    ht3 = p_h.tile([128, 3 * GROUP], f8, tag=f"hp{layer}")
                    outs = [ht3[:, i * GROUP:(i + 1) * GROUP]
                            for i in range(3)]
                    return outs, ht3[:].rearrange("p (t b) -> p t b", t=3), \
                        ht3[:]
                if bigps:
                    ht3 = p_h.tile([128, 3 * GROUP], adt, tag=f"hb{layer}")
                    outs = [ht3[:, i * GROUP:(i + 1) * GROUP]
                            for i in range(3)]
                    return outs, None, ht3[:]
                outs = []
                for i in range(3):
                    hti = p_h.tile([128, GROUP], adt, tag=f"h{i}")
                    outs.append(hti[:])
                return outs, None, None

            def relu_layer(pst, out3, layer):
                """Whole-layer bias+relu, split across ACT and DVE."""
                if "tinyrelu" in probe:
                    nc.scalar.activation(out3[:, :16], pst[:, :16], Relu,
                                         bias=bt[layer])
                    return
                cut = cfg.get("rcut", 768)
                spans = ([("A", 0, cut), ("V", cut, 3 * GROUP)]
                         if layer % 2 == 0
                         else [("V", 0, cut), ("A", cut, 3 * GROUP)])
                for e, lo, hi in spans:
                    if e == "A":
                        nc.scalar.activation(out3[:, lo:hi], pst[:, lo:hi],
                                             Relu, bias=bt[layer])
                    else:
                        nc.vector.tensor_scalar(
                            out=out3[:, lo:hi], in0=pst[:, lo:hi],
                            scalar1=bt[layer], scalar2=0.0,
                            op0=mybir.AluOpType.add, op1=mybir.AluOpType.max)

            for layer in range(4):
                houts, h3_next, hfull = new_h(layer)
                pst = None
                if bigps:
                    pst = p_ph.tile([128, 3 * GROUP], f32, tag="ph")

                def psum_for(i):
                    if bigps:
                        return pst[:, i * GROUP:(i + 1) * GROUP]
                    psi = p_ph.tile([128, GROUP], f32, tag="ph")
                    return psi[:]

                hn = [None, None, None]
                if layer in fp8set:
                    assert h3 is not None
                    for i in iperm:
                        ps = psum_for(i)
                        plist = pair_by_li[(layer, i)]
                        for bi, (pi, j0) in enumerate(plist):
                            nc.tensor.matmul(
                                ps[:],
                                lhsT=wp8[:, pi * 256:(pi + 1) * 256]
                                    .rearrange("p (t m) -> p t m", t=2),
                                rhs=h3[:, j0:j0 + 2, :],
                                start=(bi == 0),
                                stop=(bi == len(plist) - 1),
                                perf_mode=DR)
                        if not bigps:
                            relu_move(RELU_ENG[layer][i], houts[i], ps, layer)
                        hn[i] = houts[i]
                    if bigps:
                        relu_layer(pst[:], hfull, layer)
                    h, h3 = hn, h3_next
                    continue
                if cfg.get("pack") and layer in (1, 3):
                    assert not bigps
                    # Layers with in-stacking S_B have two K=64 blocks (only
                    # one node of in-tile 0 feeds them).  Run them as two
                    # concurrent 64x128 row tiles, then the full blocks.
                    ko = woff[layer * 3]
                    ps0 = p_ph.tile([128, GROUP], f32, tag="ph")
                    ps1 = p_ph.tile([128, GROUP], f32, tag="ph")
                    ps2 = p_ph.tile([128, GROUP], f32, tag="ph")
                    kk = lambda i, bi: woff[layer * 3 + i] + bi
                    w_ = lambda k, lo, hi: wtile[lo:hi,
                                                 k * 128:(k + 1) * 128]
                    nc.tensor.matmul(ps0[:], lhsT=w_(kk(0, 0), 0, 64),
                                     rhs=h[0][0:64, :], start=True,
                                     stop=False, tile_position=(0, 0))
                    nc.tensor.matmul(ps2[:], lhsT=w_(kk(2, 0), 64, 128),
                                     rhs=h[0][64:128, :], start=True,
                                     stop=False, tile_position=(64, 0))
                    nc.tensor.matmul(ps0[:], lhsT=wt[kk(0, 1)], rhs=h[1][:],
                                     start=False, stop=True)
                    nc.tensor.matmul(ps1[:], lhsT=wt[kk(1, 0)], rhs=h[1][:],
                                     start=True, stop=False)
                    nc.tensor.matmul(ps1[:], lhsT=wt[kk(1, 1)], rhs=h[2][:],
                                     start=False, stop=True)
                    nc.tensor.matmul(ps2[:], lhsT=wt[kk(2, 1)], rhs=h[2][:],
                                     start=False, stop=True)
                    for i, ps in ((0, ps0), (1, ps1), (2, ps2)):
                        relu_move(RELU_ENG[layer][i], houts[i], ps, layer)
                        hn[i] = houts[i]
                    h, h3 = hn, h3_next
                    continue
                for i in iperm:
                    k = woff[layer * 3 + i]
                    ps = p_ph.tile([128, GROUP], f32, tag="ph")
                    js = BLOCK_PLAN[layer][i]
                    for bi, j in enumerate(js):
                        nc.tensor.matmul(
                            ps[:],
                            lhsT=wt[k],
                            rhs=h[j][:],
                            start=(bi == 0),
                            stop=(bi == len(js) - 1),
                        )
                        k += 1
                    relu_move(RELU_ENG[layer][i], houts[i], ps, layer)
                    hn[i] = houts[i]
                h, h3 = hn, h3_next

            if "nofc" in probe:
                # Probe mode: drop the residual/fc/sigmoid tail; store h[0]
                # directly so the pipeline still drains to DRAM.
                sig = p_sig.tile([NN, GROUP], f32, tag="sig")
                nc.vector.tensor_copy(out=sig[:], in_=h[0][:NN, :])
                nc.sync.dma_start(y_ap[g], sig[:])
                return first_t, None

            # Residual + fc heads: logits[n, b] accumulate in PSUM [6, 512]
            # with the tiny fc weights stationary (cheap weight loads, full
            # N=512 streams), then sigmoid (+bias) and a strided store
            # straight to the batch-major DRAM layout.
            psfc = p_pot.tile([NN, GROUP], f32, tag="ot")
            first_bm = None
            if cfg.get("fcres"):
                # Fold the residual into the fc head: logits = fc^T h + fc^T x
                # (6 cheap matmuls, no elementwise adds).
                for i in range(3):
                    mi = nc.tensor.matmul(psfc[:], lhsT=fct[i], rhs=h[i][:],
                                          start=(i == 0), stop=False)
                    if first_bm is None:
                        first_bm = mi
                    nc.tensor.matmul(psfc[:], lhsT=fct[i], rhs=xs[i][:],
                                     start=False, stop=(i == 2))
            else:
                for i in range(3):
                    ri = p_r.tile([128, GROUP], adt, tag=f"r{i}")
                    eng = nc.vector if ADD_ENG[i] == "V" else nc.gpsimd
                    eng.tensor_add(out=ri[:], in0=h[i][:], in1=xs[i][:])
                    mi = nc.tensor.matmul(
                        psfc[:],
                        lhsT=fct[i],
                        rhs=ri[:],
                        start=(i == 0),
                        stop=(i == 2),
                    )
                    if first_bm is None:
                        first_bm = mi
            sig = p_sig.tile([NN, GROUP], f32, tag="sig")
            nc.scalar.activation(sig[:], psfc[:], Sigmoid, bias=fcbt[:])
            # Store node-major [6, 512] contiguously; the host un-permutes.
            nc.sync.dma_start(y_ap[g], sig[:])
            return first_t, first_bm

        from concourse.tile_rust import add_dep_helper

        def run_groups():
            prev_bm = None
            for g in range(N_GROUPS):
                first_t, first_bm = group_body(g)
                if (prev_bm is not None and cfg.get("cluster", False)
                        and first_t is not None):
                    add_dep_helper(first_t.ins, prev_bm.ins, sync=False,
                                   reason="cluster transpose-mode runs")
                prev_bm = first_bm

        if repeats == 1:
            run_groups()
        else:
            hint = (mybir.EngineType.PE, mybir.EngineType.Activation,
                    mybir.EngineType.DVE, mybir.EngineType.SP,
                    mybir.EngineType.Pool)
            with tc.For_i(0, repeats, hint_engines=hint,
                          staggered_reset=cfg.get("stag", False)):
                run_groups()

    nc.compile()
    return nc


class Runner:
    """Compiled program + cached jitted PJRT executable over the 8 cores.

    Mirrors concourse.bass2jax.run_bass_via_pjrt, but keeps the jitted
    callable and accepts device-resident inputs so repeated timed calls do
    not re-trace or re-transfer."""

    def __init__(self, nc):
        import jax
        import numpy as _np
        from jax.sharding import Mesh, PartitionSpec, NamedSharding
        from jax.experimental.shard_map import shard_map
        import concourse.mybir as mybir
        from concourse import bass2jax

        bass2jax.install_neuronx_cc_hook()
        self.nc = nc
        assert nc.dbg_addr is None
        partition_name = (nc.partition_id_tensor.name
                          if nc.partition_id_tensor else None)

        in_names, out_names, out_avals, zero_outs = [], [], [], []
        for alloc in nc.m.functions[0].allocations:
            if not isinstance(alloc, mybir.MemoryLocationSet):
                continue
            name = alloc.memorylocations[0].name
            if alloc.kind == "ExternalInput":
                if name == partition_name:
                    continue
                in_names.append(name)
            elif alloc.kind == "ExternalOutput":
                shape = tuple(alloc.tensor_shape)
                dtype = mybir.dt.np(alloc.dtype)
                out_names.append(name)
                out_avals.append(jax.core.ShapedArray(shape, dtype))
                zero_outs.append(_np.zeros(shape, dtype))
        self.in_names = list(in_names)
        self.out_names = out_names
        self.out_avals = out_avals
        self.zero_outs = zero_outs
        n_params = len(in_names)
        n_outs = len(out_avals)
        all_in_names = in_names + out_names
        if partition_name is not None:
            all_in_names = all_in_names + [partition_name]

        def _body(*args):
            operands = list(args)
            if partition_name is not None:
                operands.append(bass2jax.partition_id_tensor())
            outs = bass2jax._bass_exec_p.bind(
                *operands,
                out_avals=tuple(out_avals),
                in_names=tuple(all_in_names),
                out_names=tuple(out_names),
                lowering_input_output_aliases=(),
                sim_require_finite=True,
                sim_require_nnan=True,
                nc=nc,
            )
            return tuple(outs)

        devices = jax.devices()[:N_CORES]
        self.mesh = Mesh(_np.asarray(devices), ("core",))
        self.sharding = NamedSharding(self.mesh, PartitionSpec("core"))
        in_specs = (PartitionSpec("core"),) * (n_params + n_outs)
        out_specs = (PartitionSpec("core"),) * n_outs
        self.jitted = jax.jit(
            shard_map(_body, mesh=self.mesh, in_specs=in_specs,
                      out_specs=out_specs, check_rep=False),
            keep_unused=True,
        )
        self._jax = jax

    def put_inputs(self, in_maps):
        """in_maps: list of N_CORES dicts name->np.  Returns device arrays."""
        import numpy as _np
        concat = [
            _np.concatenate([_np.asarray(m[name]) for m in in_maps], axis=0)
            for name in self.in_names
        ]
        dev = [self._jax.device_put(a, self.sharding) for a in concat]
        # The zero "output operands" are never read by the NEFF (no
        # input/output aliasing is declared); upload them once and reuse.
        self._zeros_dev = [
            self._jax.device_put(
                self._jax.numpy.zeros((N_CORES * z.shape[0], *z.shape[1:]),
                                      z.dtype),
                self.sharding)
            for z in self.zero_outs
        ]
        return dev

    def run(self, dev_inputs):
        outs = self.jitted(*dev_inputs, *self._zeros_dev)
        outs = [self._jax.block_until_ready(o) for o in outs]
        return {
            name: outs[i]
            for i, name in enumerate(self.out_names)
        }


_RUNNERS = {}


def get_runner(repeats: int = 1, cfg: dict | None = None) -> Runner:
    key = (repeats, tuple(sorted((cfg or {}).items())))
    if key not in _RUNNERS:
        _RUNNERS[key] = Runner(build_program(repeats, cfg))
    return _RUNNERS[key]


def _make_in_maps(inputs, wdtype="f32r", xdtype=None):
    if xdtype is None:
        xdtype = wdtype if wdtype != "f32r" else "f32r"
    x = np.ascontiguousarray(np.asarray(inputs["x"], np.float32))
    assert x.shape == (BATCH, NN, FEAT)
    consts = build_consts(
        W=[np.asarray(inputs[f"W{i+1}"], np.float32) for i in range(4)],
        b=[np.asarray(inputs[f"b{i+1}"], np.float32) for i in range(4)],
        fc_w=np.asarray(inputs["fc_w"], np.float32),
        fc_b=np.asarray(inputs["fc_b"], np.float32),
    )
    if wdtype == "bf16":
        import ml_dtypes
        consts["wblk"] = consts["wblk"].astype(ml_dtypes.bfloat16)
        consts["fcw"] = consts["fcw"].astype(ml_dtypes.bfloat16)
    if xdtype == "bf16":
        import ml_dtypes
        x = x.astype(ml_dtypes.bfloat16)
        consts["eye128"] = consts["eye128"].astype(ml_dtypes.bfloat16)
    x_sh = x.reshape(N_CORES, PER_CORE, NN * FEAT)
    return [{"x": x_sh[c], **consts} for c in range(N_CORES)]


FP8_SETS = {None: frozenset(), "l234": frozenset({1, 2, 3}),
            "all": frozenset({0, 1, 2, 3})}


def _maps_for_cfg(inputs, cfg):
    cfg = dict(cfg or {})
    xdt = cfg.get("xdtype", "f32r")
    wdt = cfg.get("wdtype", xdt)
    maps = _make_in_maps(inputs, wdtype=wdt, xdtype=xdt)
    fp8set = FP8_SETS[cfg.get("fp8")]
    if fp8set:
        import ml_dtypes
        consts = build_consts(
            W=[np.asarray(inputs[f"W{i+1}"], np.float32) for i in range(4)],
            b=[np.asarray(inputs[f"b{i+1}"], np.float32) for i in range(4)],
            fc_w=np.asarray(inputs["fc_w"], np.float32),
            fc_b=np.asarray(inputs["fc_b"], np.float32),
        )
        pairs = _pair_plan(fp8set)
        wpair = np.zeros((len(pairs), 128, 256), np.float32)
        for pi, (_l, _i, _j0, (ka, kb)) in enumerate(pairs):
            if ka is not None:
                wpair[pi, :, :128] = consts["wblk"][ka]
            if kb is not None:
                wpair[pi, :, 128:] = consts["wblk"][kb]
        wp8 = wpair.astype(ml_dtypes.float8_e4m3)
        for m in maps:
            m["wpair8"] = wp8
    if cfg.get("hostt"):
        import ml_dtypes
        x = np.ascontiguousarray(np.asarray(inputs["x"], np.float32))
        # [C, NG, 512, 3, 128] -> feature-major [C, NG, 128(p), 3(j), 512(b)]
        xt = np.ascontiguousarray(
            x.reshape(N_CORES, N_GROUPS, GROUP, 3, 128)
            .transpose(0, 1, 4, 3, 2)).reshape(N_CORES, N_GROUPS, 128,
                                               3 * GROUP)
        xt_bf = xt.astype(ml_dtypes.bfloat16)
        xt_f8 = (xt.astype(ml_dtypes.float8_e4m3)
                 if 0 in fp8set else None)
        for c, m in enumerate(maps):
            m.pop("x", None)
            m["xt"] = xt_bf[c]
            if xt_f8 is not None:
                m["xt8"] = xt_f8[c]
    return maps


def unpack_y(y_raw: np.ndarray) -> np.ndarray:
    """Device output [N_CORES * N_GROUPS, NN, GROUP] -> [BATCH, NN]."""
    y = y_raw.reshape(N_CORES, N_GROUPS, NN, GROUP)
    return np.ascontiguousarray(
        y.transpose(0, 1, 3, 2).reshape(BATCH, NN))


# Best measured configuration (see session notes): bf16 end-to-end with the
# XBAR DMA-transpose input path.
DEFAULT_CFG = {"eng": "base", "xdtype": "bf16", "dmat": True}


def kernel(**inputs) -> np.ndarray:
    runner = get_runner(repeats=1, cfg=DEFAULT_CFG)
    dev = runner.put_inputs(_maps_for_cfg(inputs, DEFAULT_CFG))
    out = runner.run(dev)
    return unpack_y(np.asarray(out["y"]))



# revision 39
# speedup vs baseline: 1.5530x; 1.5530x over previous
"""Trainium2 Bass kernel for the 6-node GCN classification model.

Math: each GCN layer is h' = relu(A @ h @ W^T + b) on [B, 6, 64], where A is
the 6x6 normalized adjacency (with self loops; fill=1.0 for layers 1-2,
fill=2.0 for layers 3-4).  With the 6 nodes stacked in pairs on the 128 SBUF
partitions (2 nodes x 64 features), the fused per-layer operator A (x) W is a
384x384 block matrix; each nonzero 128x128 block becomes one TensorE matmul
accumulated in PSUM.  Node pairings alternate between two stackings chosen so
the total block count over the 4 layers is the provable minimum (26).

Pipeline per 512-batch group, fully fused in SBUF (x is read from HBM exactly
once, only the [B, 6] sigmoid output is written back):
  DMA x (batch-major, contiguous) -> PE transpose to feature-major stacked
  -> 4 x (block matmuls f32r -> ACT bias+ReLU) -> DVE residual add
  -> fc-head matmuls -> ACT sigmoid -> PE transpose back -> DMA out.

Sharding: pure data parallel over the batch dim across the 8 NeuronCores.
"""

import math
from contextlib import ExitStack, nullcontext as _nullctx

import numpy as np

N_CORES = 8
BATCH = 131072
PER_CORE = BATCH // N_CORES  # 16384
NN = 6
FEAT = 64
GROUP = 512
N_GROUPS = PER_CORE // GROUP  # 32

SRC = [1, 2, 0, 2, 1, 3, 2, 4, 3, 5, 3, 4]
DST = [0, 0, 1, 1, 2, 2, 3, 3, 4, 4, 5, 5]

# Node pair stackings per layer boundary (chain start == chain end so the
# residual/fc read the same stacking the input transposes produce).
S_A = [(0, 1), (2, 3), (4, 5)]
S_B = [(0, 5), (1, 2), (3, 4)]
CHAIN = [S_A, S_B, S_A, S_B, S_A]  # layer l maps CHAIN[l] -> CHAIN[l+1]
_IPERMS = [(0, 1, 2), (0, 2, 1), (1, 0, 2), (1, 2, 0), (2, 0, 1), (2, 1, 0)]

# Engine assignments for the post-matmul elementwise ops.  GPSIMD (Pool)
# cannot access PSUM on TRN2, so the PSUM->SBUF relu moves and xt copies can
# only go to ACT (A) / DVE (V); Pool (P) can take SBUF->SBUF residual adds.
ENG_TABLES = {
    # baseline: ACT does everything except one relu per early layer on DVE
    "base": dict(
        relu=[("A", "A", "V"), ("A", "A", "V"), ("A", "A", "V"),
              ("A", "A", "A")],
        cp=("V", "V", "V"), add=("V", "V", "V")),
    # even ACT/DVE split, residual adds on Pool
    "split": dict(
        relu=[("A", "V", "A"), ("V", "A", "V"), ("A", "V", "A"),
              ("V", "A", "V")],
        cp=("V", "A", "V"), add=("P", "P", "P")),
    # even ACT/DVE split, residual adds on DVE
    "splitv": dict(
        relu=[("A", "V", "A"), ("V", "A", "V"), ("A", "V", "A"),
              ("V", "A", "V")],
        cp=("V", "A", "V"), add=("V", "V", "V")),
    # fp8+dmat balance: ACT carries 5 relus + sigmoid, DVE 7 relus, Pool adds
    "split8": dict(
        relu=[("A", "V", "V"), ("A", "V", "A"), ("V", "A", "V"),
              ("A", "V", "V")],
        cp=("V", "A", "V"), add=("P", "P", "P")),
}


def _gcn_A(fill: float) -> np.ndarray:
    """Dense [6, 6] aggregation matrix A[dst, src] incl. weighted self loops."""
    src = SRC + list(range(NN))
    dst = DST + list(range(NN))
    w = [1.0] * len(SRC) + [fill] * NN
    deg = np.zeros(NN, np.float64)
    for s, d, ww in zip(src, dst, w):
        deg[d] += ww
    dinv = np.where(deg > 0, 1.0 / np.sqrt(deg), 0.0)
    A = np.zeros((NN, NN), np.float64)
    for s, d, ww in zip(src, dst, w):
        A[d, s] += dinv[s] * ww * dinv[d]
    return A


def _block_plan():
    """Static plan: for each layer, the nonzero (out_tile, in_tile) blocks.

    Returns [layer][out_tile] -> list of in_tile indices, using the support of
    A (same for both fill values)."""
    S = np.zeros((NN, NN), bool)
    for s, d in zip(SRC, DST):
        S[d, s] = True
    for i in range(NN):
        S[i, i] = True
    plan = []
    for layer in range(4):
        inp, outp = CHAIN[layer], CHAIN[layer + 1]
        lplan = []
        for (n0, n1) in outp:
            js = []
            for j, (m0, m1) in enumerate(inp):
                if S[n0, m0] or S[n0, m1] or S[n1, m0] or S[n1, m1]:
                    js.append(j)
            lplan.append(js)
        plan.append(lplan)
    return plan


BLOCK_PLAN = _block_plan()
N_BLOCKS = sum(len(js) for lp in BLOCK_PLAN for js in lp)  # 26


def _pair_plan(fp8set):
    """DoubleRow pair schedule for the fp8 layers.

    Each entry: (layer, out_tile, rhs_j0, (blkA|None, blkB|None)) meaning one
    DoubleRow matmul with rhs = h[j0], h[j0+1] and the two stationary K-tiles
    holding A-scaled weight blocks (None = zero block).  blkX indexes into
    the flat wblk ordering used by build_consts."""
    woff = [0]
    for layer in range(4):
        for i in range(3):
            woff.append(woff[-1] + len(BLOCK_PLAN[layer][i]))
    plan = []
    for layer in sorted(fp8set):
        for i in range(3):
            js = BLOCK_PLAN[layer][i]
            k0 = woff[layer * 3 + i]
            ks = {j: k0 + bi for bi, j in enumerate(js)}
            if js == [0, 1] or js == [1, 2]:
                plan.append((layer, i, js[0], (ks[js[0]], ks[js[1]])))
            elif js == [0, 1, 2]:
                plan.append((layer, i, 0, (ks[0], ks[1])))
                plan.append((layer, i, 1, (None, ks[2])))
            elif js == [0, 2]:
                plan.append((layer, i, 0, (ks[0], None)))
                plan.append((layer, i, 1, (None, ks[2])))
            else:
                raise AssertionError(js)
    return plan


def _exec_plan():
    """Improved fp8 matmul plan (all 4 layers fp8).

    Returns (ops, pairs): ops[(layer, i)] = list of
    ('dr', j0, step, pair_idx) | ('mm', j, blk_idx).  A 'dr' is one DoubleRow
    matmul with rhs slots (j0, j0+step) and stationary pair pair_idx in the
    wpair8 tensor; an 'mm' is a plain fp8 matmul on flat block blk_idx of
    wblk8.  Versus the legacy _pair_plan this never issues a half-empty
    DoubleRow: [0,2] inputs use a stride-2 rhs AP and [0,1,2] runs as one
    DoubleRow plus one cheap normal matmul.
    """
    woff = [0]
    for layer in range(4):
        for i in range(3):
            woff.append(woff[-1] + len(BLOCK_PLAN[layer][i]))
    ops = {}
    pairs = []
    for layer in range(4):
        for i in range(3):
            js = BLOCK_PLAN[layer][i]
            k0 = woff[layer * 3 + i]
            ks = {j: k0 + bi for bi, j in enumerate(js)}
            lops = []
            if js == [0, 1] or js == [1, 2]:
                lops.append(("dr", js[0], 1, len(pairs)))
                pairs.append((ks[js[0]], ks[js[1]]))
            elif js == [0, 2]:
                lops.append(("dr", 0, 2, len(pairs)))
                pairs.append((ks[0], ks[2]))
            elif js == [0, 1, 2]:
                lops.append(("dr", 0, 1, len(pairs)))
                pairs.append((ks[0], ks[1]))
                lops.append(("mm", 2, ks[2]))
            else:
                raise AssertionError(js)
            ops[(layer, i)] = lops
    return ops, pairs


EXEC_OPS, EXEC_PAIRS = _exec_plan()


def build_consts(W, b, fc_w, fc_b):
    """Host-side constant tensors fed to the device as DRAM inputs.

    W: list of 4 [64, 64] arrays; b: list of 4 [64]; fc_w [6, 64]; fc_b [6].
    """
    A = [_gcn_A(1.0), _gcn_A(1.0), _gcn_A(2.0), _gcn_A(2.0)]
    wblk = np.zeros((N_BLOCKS, 128, 128), np.float32)
    k = 0
    for layer in range(4):
        inp, outp = CHAIN[layer], CHAIN[layer + 1]
        Wt = W[layer].T.astype(np.float64)  # [f, g] = W[g, f]
        for i, (n0, n1) in enumerate(outp):
            for j in BLOCK_PLAN[layer][i]:
                m0, m1 = inp[j]
                blk = np.zeros((128, 128), np.float64)
                for dj, m in enumerate((m0, m1)):
                    for do, n in enumerate((n0, n1)):
                        a = A[layer][n, m]
                        if a != 0.0:
                            blk[dj * 64:(dj + 1) * 64, do * 64:(do + 1) * 64] = a * Wt
                wblk[k] = blk.astype(np.float32)
                k += 1
    assert k == N_BLOCKS

    bias = np.zeros((4, 128), np.float32)
    for layer in range(4):
        bias[layer] = np.tile(b[layer], 2)

    fcw = np.zeros((3, 128, NN), np.float32)
    for i, (n0, n1) in enumerate(CHAIN[4]):
        for do, n in enumerate((n0, n1)):
            fcw[i, do * 64:(do + 1) * 64, n] = fc_w[n]

    return {
        "wblk": wblk,
        "bias": bias,
        "fcw": fcw,
        "fcb": fc_b.astype(np.float32).reshape(NN, 1),
        "eye128": np.eye(128, dtype=np.float32),
    }


def build_program(repeats: int = 1, cfg: dict | None = None):
    """Build + schedule + compile the Bass/Tile program. Returns nc."""
    import concourse.tile as tile
    import concourse.mybir as mybir
    from concourse import bacc

    cfg = dict(cfg or {})
    bufs_xb = cfg.get("xb", 3)
    bufs_xs = cfg.get("xs", 3)
    bufs_h = cfg.get("h", 4)
    bufs_r = cfg.get("r", 2)
    bufs_px = cfg.get("px", 2)
    bufs_ph = cfg.get("ph", 5)
    bufs_pfc = cfg.get("pfc", 1)
    bufs_pot = cfg.get("pot", 1)
    xt_in_ph = cfg.get("xt_in_ph", False)
    ot_in_pfc = cfg.get("ot_in_pfc", False)
    xdt_name = cfg.get("xdtype", "f32r")
    wdt_name = cfg.get("wdtype", xdt_name)
    dmat = cfg.get("dmat", False)  # XBAR DMA-transpose input path (bf16 only)
    hostt = cfg.get("hostt", False)  # host pre-transposed x upload
    if dmat or hostt:
        assert xdt_name == "bf16"
        bufs_ph = cfg.get("ph", 7)  # px pool unused -> spend its PSUM banks
    if cfg.get("bigps"):
        bufs_ph = cfg.get("ph", 2)  # [128, 3*GROUP] tiles = 3 banks each
    fp8set = FP8_SETS[cfg.get("fp8")]
    pairs = _pair_plan(fp8set) if fp8set else []
    ENG = ENG_TABLES[cfg.get("eng", "base")]
    RELU_ENG, CP_ENG, ADD_ENG = ENG["relu"], ENG["cp"], ENG["add"]

    f32 = mybir.dt.float32
    f32r = mybir.dt.float32r
    _DT = {"f32r": mybir.dt.float32r, "bf16": mybir.dt.bfloat16}
    wdt = _DT[wdt_name]
    adt = _DT[xdt_name]  # on-chip dtype of x and of the layer activations
    Relu = mybir.ActivationFunctionType.Relu
    Sigmoid = mybir.ActivationFunctionType.Sigmoid
    Copy = mybir.ActivationFunctionType.Copy
    f8 = mybir.dt.float8e4
    DR = mybir.MatmulPerfMode.DoubleRow

    nc = bacc.Bacc("TRN2", target_bir_lowering=False, debug=False,
                   num_devices=N_CORES)

    if hostt:
        xt_ap = nc.dram_tensor("xt", [N_GROUPS, 128, 3 * GROUP], adt,
                               kind="ExternalInput").ap()
        if 0 in fp8set:
            xt8_ap = nc.dram_tensor("xt8", [N_GROUPS, 128, 3 * GROUP],
                                    mybir.dt.float8e4,
                                    kind="ExternalInput").ap()
    else:
        x_ap = nc.dram_tensor("x", [PER_CORE, NN * FEAT], adt,
                              kind="ExternalInput").ap()
    y_ap = nc.dram_tensor("y", [N_GROUPS, NN, GROUP], f32,
                          kind="ExternalOutput").ap()
    wblk_ap = nc.dram_tensor("wblk", [N_BLOCKS, 128, 128], wdt,
                             kind="ExternalInput").ap()
    bias_ap = nc.dram_tensor("bias", [4, 128], f32,
                             kind="ExternalInput").ap()
    fcw_ap = nc.dram_tensor("fcw", [3, 128, NN], wdt,
                            kind="ExternalInput").ap()
    fcb_ap = nc.dram_tensor("fcb", [NN, 1], f32, kind="ExternalInput").ap()
    eye128_ap = nc.dram_tensor("eye128", [128, 128], adt,
                               kind="ExternalInput").ap()
    if pairs:
        wp8_ap = nc.dram_tensor("wpair8", [len(pairs), 128, 256], f8,
                                kind="ExternalInput").ap()

    SB = GROUP // 128  # 4 batch sub-tiles per group

    with tile.TileContext(nc) as tc, ExitStack() as ctx:
        cpool = ctx.enter_context(tc.tile_pool(name="consts", bufs=1))
        p_xb = ctx.enter_context(tc.tile_pool(name="xb", bufs=bufs_xb))
        p_xs = ctx.enter_context(tc.tile_pool(name="xs", bufs=bufs_xs))
        p_h = ctx.enter_context(tc.tile_pool(name="h", bufs=bufs_h))
        p_r = ctx.enter_context(tc.tile_pool(name="r", bufs=bufs_r))
        p_sig = ctx.enter_context(tc.tile_pool(name="sig", bufs=2))
        p_ob = ctx.enter_context(tc.tile_pool(name="ob", bufs=2))
        p_ph = ctx.enter_context(tc.tile_pool(name="ph", bufs=bufs_ph, space="PSUM"))
        p_px = p_ph if xt_in_ph else ctx.enter_context(
            tc.tile_pool(name="px", bufs=bufs_px, space="PSUM"))
        p_pot = ctx.enter_context(
            tc.tile_pool(name="pot", bufs=bufs_pot, space="PSUM"))
        eye128 = cpool.tile([128, 128], adt, tag="eye128")
        nc.sync.dma_start(eye128[:], eye128_ap[:])
        btile = cpool.tile([128, 4], f32, tag="bias")
        nc.sync.dma_start(btile[:], bias_ap.rearrange("l p -> p l"))
        bt = [btile[:, layer:layer + 1] for layer in range(4)]
        ftile = cpool.tile([128, 3 * NN], wdt, tag="fcw")
        nc.sync.dma_start(ftile[:].rearrange("p (i n) -> p i n", i=3),
                          fcw_ap.rearrange("i p n -> p i n"))
        fct = [ftile[:, i * NN:(i + 1) * NN] for i in range(3)]
        fcbt = cpool.tile([NN, 1], f32, tag="fcb")
        nc.sync.dma_start(fcbt[:], fcb_ap[:])
        def load_xb(g):
            xb = p_xb.tile([128, SB * NN * FEAT], adt, tag="xb")
            nc.sync.dma_start(
                xb[:].rearrange("p (s f) -> p s f", s=SB),
                x_ap[g * GROUP:(g + 1) * GROUP, :].rearrange(
                    "(s p) f -> p s f", p=128),
            )
            return xb

        def load_xs_t(g):
            # XBAR DMA transpose: [512, 384] bf16 rows -> feature-major
            # [128, 3, 512] (chunk j holds features 128j..128j+127, which is
            # exactly the S_A node-pair stacking).  Replaces 12 PE transposes
            # and 3 PSUM->SBUF copies per group.
            xst = p_xs.tile([128, 3 * GROUP], adt, tag="xs")
            nc.sync.dma_start_transpose(
                xst[:].rearrange("p (j b) -> p j b", j=3),
                x_ap[g * GROUP:(g + 1) * GROUP, :])
            return xst

        def load_xs_host(g):
            # Host pre-transposed x: plain contiguous DMA per group (bf16 for
            # layer-1/residual, plus fp8 copy when layer 1 runs DoubleRow).
            xst = p_xs.tile([128, 3 * GROUP], adt, tag="xs")
            nc.sync.dma_start(xst[:], xt_ap[g])
            x8 = None
            if 0 in fp8set:
                x8 = p_xs.tile([128, 3 * GROUP], f8, tag="xs8")
                nc.sync.dma_start(x8[:], xt8_ap[g])
            return (xst, x8)

        # The first groups' x tiles go before the big weight DMAs so the
        # transposes can start while the weights stream in.  (Single-pass
        # builds only: under a repeat loop these DMAs would not replay.)
        load_x_pre = (load_xs_host if hostt
                      else (load_xs_t if dmat else load_xb))
        xb_pre = ({g: load_x_pre(g) for g in range(min(2, N_GROUPS))}
                  if repeats == 1 else {})

        # Block weights in two DMAs (layer-0 blocks first so the first
        # group's matmuls can start before the rest of the weights land).
        nb0 = sum(len(js) for js in BLOCK_PLAN[0])
        wtile = cpool.tile([128, N_BLOCKS * 128], wdt, tag="wblk")
        nc.sync.dma_start(
            wtile[:, :nb0 * 128].rearrange("p (k f) -> p k f", k=nb0),
            wblk_ap[:nb0].rearrange("k p f -> p k f"))
        nc.sync.dma_start(
            wtile[:, nb0 * 128:].rearrange("p (k f) -> p k f", k=N_BLOCKS - nb0),
            wblk_ap[nb0:].rearrange("k p f -> p k f"))
        wt = [wtile[:, k * 128:(k + 1) * 128] for k in range(N_BLOCKS)]

        if pairs:
            wp8 = cpool.tile([128, len(pairs) * 256], f8, tag="wpair8")
            nc.sync.dma_start(
                wp8[:].rearrange("p (k f) -> p k f", k=len(pairs)),
                wp8_ap.rearrange("k p f -> p k f"))
            pair_by_li = {}
            for pi, (pl, i, j0, _ks) in enumerate(pairs):
                pair_by_li.setdefault((pl, i), []).append((pi, j0))


        probe = cfg.get("probe", "")

        def relu_move(e, ht, ps, layer):
            if "tinyrelu" in probe:
                # Probe mode: keep the dataflow edges but shrink the
                # elementwise work to ~nothing (timing only, wrong numerics).
                nc.scalar.activation(ht[:, :16], ps[:, :16], Relu,
                                     bias=bt[layer])
                return
            if cfg.get("relu2"):
                # Halve the relu latency on the critical path: ACT and DVE
                # each process half the tile concurrently.
                half = GROUP // 2
                nc.scalar.activation(ht[:, :half], ps[:, :half], Relu,
                                     bias=bt[layer])
                nc.vector.tensor_scalar(out=ht[:, half:], in0=ps[:, half:],
                                        scalar1=bt[layer], scalar2=0.0,
                                        op0=mybir.AluOpType.add,
                                        op1=mybir.AluOpType.max)
                return
            if e == "A":
                nc.scalar.activation(ht[:], ps[:], Relu, bias=bt[layer])
            else:
                eng = nc.vector if e == "V" else nc.gpsimd
                eng.tensor_scalar(out=ht[:], in0=ps[:], scalar1=bt[layer],
                                  scalar2=0.0, op0=mybir.AluOpType.add,
                                  op1=mybir.AluOpType.max)

        def group_body(g):
            first_t = None
            xs8t_loaded = None
            if hostt:
                pre = xb_pre.pop(g, None)
                xst, xs8t_loaded = pre if pre is not None else load_xs_host(g)
                xs = [xst[:, j * GROUP:(j + 1) * GROUP] for j in range(3)]
            elif dmat:
                xst = xb_pre.pop(g, None)
                if xst is None:
                    xst = load_xs_t(g)
                xs = [xst[:, j * GROUP:(j + 1) * GROUP] for j in range(3)]
            else:
                # Load [512, 384] rows batch-major: partition = batch % 128.
                xb = xb_pre.pop(g, None)
                if xb is None:
                    xb = load_xb(g)
                # Transpose to feature-major stacked (pairs = CHAIN[0]).
                xs = []
                xts = []
                for j in range(3):
                    xt = p_px.tile([128, GROUP], adt,
                                   tag="ph" if xt_in_ph else "xt")
                    for s in range(SB):
                        ti = nc.tensor.transpose(
                            xt[:, s * 128:(s + 1) * 128],
                            xb[:, s * NN * FEAT + j * 128:
                               s * NN * FEAT + (j + 1) * 128],
                            eye128[:],
                        )
                        if first_t is None:
                            first_t = ti
                    xts.append(xt)
                for j in range(3):
                    xsj = p_xs.tile([128, GROUP], adt, tag=f"xs{j}")
                    e = CP_ENG[j]
                    if e == "A":
                        nc.scalar.activation(xsj[:], xts[j][:], Copy)
                    else:
                        eng = nc.vector if e == "V" else nc.gpsimd
                        eng.tensor_copy(out=xsj[:], in_=xts[j][:])
                    xs.append(xsj)

            h = xs
            iperm = _IPERMS[cfg.get("iorder", 0)]
            woff = [0]
            for layer in range(4):
                for i in range(3):
                    woff.append(woff[-1] + len(BLOCK_PLAN[layer][i]))

            # h3: [p, t, b] single-tile view of the CURRENT h list, required
            # as the DoubleRow rhs of an fp8 layer.
            if 0 in fp8set:
                if xs8t_loaded is not None:
                    h3 = xs8t_loaded[:].rearrange("p (t b) -> p t b", t=3)
                else:
                    xs8t = p_xs.tile([128, 3 * GROUP], f8, tag="xs8")
                    for j in range(3):
                        if CP_ENG[j] == "A":
                            nc.scalar.activation(
                                xs8t[:, j * GROUP:(j + 1) * GROUP],
                                xs[j][:], Copy)
                        else:
                            nc.vector.tensor_copy(
                                out=xs8t[:, j * GROUP:(j + 1) * GROUP],
                                in_=xs[j][:])
                    h3 = xs8t[:].rearrange("p (t b) -> p t b", t=3)
            else:
                h3 = None

            bigps = cfg.get("bigps", False)

            def new_h(layer):
                """Output container for a layer: (slices, [p,t,b] view or
                None, full [128,1536] AP or None).  Single tile when the next
                layer consumes it via DoubleRow or when bigps needs span
                writes."""
                if (layer + 1) in fp8set:
                    ht3 = p_h.tile([128, 3 * GROUP], f8, tag=f"hp{layer}")
                    outs = [ht3[:, i * GROUP:(i + 1) * GROUP]
                            for i in range(3)]
                    return outs, ht3[:].rearrange("p (t b) -> p t b", t=3), \
                        ht3[:]
                if bigps:
                    ht3 = p_h.tile([128, 3 * GROUP], adt, tag=f"hb{layer}")
                    outs = [ht3[:, i * GROUP:(i + 1) * GROUP]
                            for i in range(3)]
                    return outs, None, ht3[:]
                outs = []
                for i in range(3):
                    hti = p_h.tile([128, GROUP], adt, tag=f"h{i}")
                    outs.append(hti[:])
                return outs, None, None

            def relu_layer(pst, out3, layer):
                """Whole-layer bias+relu, split across ACT and DVE."""
                if "tinyrelu" in probe:
                    nc.scalar.activation(out3[:, :16], pst[:, :16], Relu,
                                         bias=bt[layer])
                    return
                cut = cfg.get("rcut", 768)
                spans = ([("A", 0, cut), ("V", cut, 3 * GROUP)]
                         if layer % 2 == 0
                         else [("V", 0, cut), ("A", cut, 3 * GROUP)])
                for e, lo, hi in spans:
                    if e == "A":
                        nc.scalar.activation(out3[:, lo:hi], pst[:, lo:hi],
                                             Relu, bias=bt[layer])
                    else:
                        nc.vector.tensor_scalar(
                            out=out3[:, lo:hi], in0=pst[:, lo:hi],
                            scalar1=bt[layer], scalar2=0.0,
                            op0=mybir.AluOpType.add, op1=mybir.AluOpType.max)

            for layer in range(4):
                houts, h3_next, hfull = new_h(layer)
                pst = None
                if bigps:
                    pst = p_ph.tile([128, 3 * GROUP], f32, tag="ph")

                def psum_for(i):
                    if bigps:
                        return pst[:, i * GROUP:(i + 1) * GROUP]
                    psi = p_ph.tile([128, GROUP], f32, tag="ph")
                    return psi[:]

                hn = [None, None, None]
                if layer in fp8set:
                    assert h3 is not None
                    for i in iperm:
                        ps = psum_for(i)
                        plist = pair_by_li[(layer, i)]
                        for bi, (pi, j0) in enumerate(plist):
                            nc.tensor.matmul(
                                ps[:],
                                lhsT=wp8[:, pi * 256:(pi + 1) * 256]
                                    .rearrange("p (t m) -> p t m", t=2),
                                rhs=h3[:, j0:j0 + 2, :],
                                start=(bi == 0),
                                stop=(bi == len(plist) - 1),
                                perf_mode=DR)
                        if not bigps:
                            relu_move(RELU_ENG[layer][i], houts[i], ps, layer)
                        hn[i] = houts[i]
                    if bigps:
                        relu_layer(pst[:], hfull, layer)
                    h, h3 = hn, h3_next
                    continue
                if cfg.get("pack") and layer in (1, 3):
                    assert not bigps
                    # Layers with in-stacking S_B have two K=64 blocks (only
                    # one node of in-tile 0 feeds them).  Run them as two
                    # concurrent 64x128 row tiles, then the full blocks.
                    ko = woff[layer * 3]
                    ps0 = p_ph.tile([128, GROUP], f32, tag="ph")
                    ps1 = p_ph.tile([128, GROUP], f32, tag="ph")
                    ps2 = p_ph.tile([128, GROUP], f32, tag="ph")
                    kk = lambda i, bi: woff[layer * 3 + i] + bi
                    w_ = lambda k, lo, hi: wtile[lo:hi,
                                                 k * 128:(k + 1) * 128]
                    nc.tensor.matmul(ps0[:], lhsT=w_(kk(0, 0), 0, 64),
                                     rhs=h[0][0:64, :], start=True,
                                     stop=False, tile_position=(0, 0))
                    nc.tensor.matmul(ps2[:], lhsT=w_(kk(2, 0), 64, 128),
                                     rhs=h[0][64:128, :], start=True,
                                     stop=False, tile_position=(64, 0))
                    nc.tensor.matmul(ps0[:], lhsT=wt[kk(0, 1)], rhs=h[1][:],
                                     start=False, stop=True)
                    nc.tensor.matmul(ps1[:], lhsT=wt[kk(1, 0)], rhs=h[1][:],
                                     start=True, stop=False)
                    nc.tensor.matmul(ps1[:], lhsT=wt[kk(1, 1)], rhs=h[2][:],
                                     start=False, stop=True)
                    nc.tensor.matmul(ps2[:], lhsT=wt[kk(2, 1)], rhs=h[2][:],
                                     start=False, stop=True)
                    for i, ps in ((0, ps0), (1, ps1), (2, ps2)):
                        relu_move(RELU_ENG[layer][i], houts[i], ps, layer)
                        hn[i] = houts[i]
                    h, h3 = hn, h3_next
                    continue
                for i in iperm:
                    k = woff[layer * 3 + i]
                    ps = p_ph.tile([128, GROUP], f32, tag="ph")
                    js = BLOCK_PLAN[layer][i]
                    for bi, j in enumerate(js):
                        nc.tensor.matmul(
                            ps[:],
                            lhsT=wt[k],
                            rhs=h[j][:],
                            start=(bi == 0),
                            stop=(bi == len(js) - 1),
                        )
                        k += 1
                    relu_move(RELU_ENG[layer][i], houts[i], ps, layer)
                    hn[i] = houts[i]
                h, h3 = hn, h3_next

            if "nofc" in probe:
                # Probe mode: drop the residual/fc/sigmoid tail; store h[0]
                # directly so the pipeline still drains to DRAM.
                sig = p_sig.tile([NN, GROUP], f32, tag="sig")
                nc.vector.tensor_copy(out=sig[:], in_=h[0][:NN, :])
                nc.sync.dma_start(y_ap[g], sig[:])
                return first_t, None

            # Residual + fc heads: logits[n, b] accumulate in PSUM [6, 512]
            # with the tiny fc weights stationary (cheap weight loads, full
            # N=512 streams), then sigmoid (+bias) and a strided store
            # straight to the batch-major DRAM layout.
            psfc = p_pot.tile([NN, GROUP], f32, tag="ot")
            first_bm = None
            if cfg.get("fcres"):
                # Fold the residual into the fc head: logits = fc^T h + fc^T x
                # (6 cheap matmuls, no elementwise adds).
                for i in range(3):
                    mi = nc.tensor.matmul(psfc[:], lhsT=fct[i], rhs=h[i][:],
                                          start=(i == 0), stop=False)
                    if first_bm is None:
                        first_bm = mi
                    nc.tensor.matmul(psfc[:], lhsT=fct[i], rhs=xs[i][:],
                                     start=False, stop=(i == 2))
            else:
                for i in range(3):
                    ri = p_r.tile([128, GROUP], adt, tag=f"r{i}")
                    eng = nc.vector if ADD_ENG[i] == "V" else nc.gpsimd
                    eng.tensor_add(out=ri[:], in0=h[i][:], in1=xs[i][:])
                    mi = nc.tensor.matmul(
                        psfc[:],
                        lhsT=fct[i],
                        rhs=ri[:],
                        start=(i == 0),
                        stop=(i == 2),
                    )
                    if first_bm is None:
                        first_bm = mi
            sig = p_sig.tile([NN, GROUP], f32, tag="sig")
            nc.scalar.activation(sig[:], psfc[:], Sigmoid, bias=fcbt[:])
            # Store node-major [6, 512] contiguously; the host un-permutes.
            nc.sync.dma_start(y_ap[g], sig[:])
            return first_t, first_bm

        from concourse.tile_rust import add_dep_helper

        def run_groups():
            prev_bm = None
            for g in range(N_GROUPS):
                first_t, first_bm = group_body(g)
                if (prev_bm is not None and cfg.get("cluster", False)
                        and first_t is not None):
                    add_dep_helper(first_t.ins, prev_bm.ins, sync=False,
                                   reason="cluster transpose-mode runs")
                prev_bm = first_bm

        if repeats == 1:
            run_groups()
        else:
            hint = (mybir.EngineType.PE, mybir.EngineType.Activation,
                    mybir.EngineType.DVE, mybir.EngineType.SP,
                    mybir.EngineType.Pool)
            with tc.For_i(0, repeats, hint_engines=hint,
                          staggered_reset=cfg.get("stag", False)):
                run_groups()

    nc.compile()
    return nc


def build_program_v2(repeats: int = 1, cfg: dict | None = None):
    """Streamlined all-fp8 build: host-pretransposed x (bf16 + fp8), the
    _exec_plan matmul schedule (12 DoubleRow + 2 plain fp8 matmuls per
    group), per-op relu engine table, sigmoid/output-DMA batched over
    `sigb` groups, optional 2-group matmul interleave (`gpair`) to reuse
    stationary weights back-to-back."""
    import concourse.tile as tile
    import concourse.mybir as mybir
    from concourse import bacc

    cfg = dict(cfg or {})
    wave = cfg.get("wave", False)
    G = cfg.get("G", GROUP)           # batch elements per group
    NSUB = G // GROUP                 # 512-wide matmul sub-slices per group
    NG = PER_CORE // G
    gdef = 2 if G == GROUP else 1     # matmul-interleave width default
    bufs_xs = cfg.get("xs", (13 if G == GROUP else 7) if wave else 3)
    bufs_h = cfg.get("h", 4 if wave else 3)
    bufs_r = cfg.get("r", 3 if wave else 2)
    sigb = cfg.get("sigb", 2 if G == GROUP else 1)
    gsub = cfg.get("gsub", gdef if cfg.get("gpair") or wave else 1)
    fcres = cfg.get("fcres", False)
    _blk = max(sigb, gsub)
    bufs_pot = cfg.get("pot", 1)
    bufs_ph = cfg.get("ph", (8 - bufs_pot * sigb * NSUB) // NSUB)
    rtab = cfg.get("rtab", "AAV AVA VAV AVA").split()
    addeng = cfg.get("addeng", "V")

    f32 = mybir.dt.float32
    bf16 = mybir.dt.bfloat16
    f8 = mybir.dt.float8e4
    Relu = mybir.ActivationFunctionType.Relu
    Sigmoid = mybir.ActivationFunctionType.Sigmoid
    DR = mybir.MatmulPerfMode.DoubleRow
    NP = len(EXEC_PAIRS)

    nc = bacc.Bacc("TRN2", target_bir_lowering=False, debug=False,
                   num_devices=N_CORES)

    xt_ap = nc.dram_tensor("xt", [NG, 128, 3 * G], bf16,
                           kind="ExternalInput").ap()
    xt8_ap = nc.dram_tensor("xt8", [NG, 128, 3 * G], f8,
                            kind="ExternalInput").ap()
    y_ap = nc.dram_tensor("y", [NG, NN, G], f32,
                          kind="ExternalOutput").ap()
    wp8_ap = nc.dram_tensor("wpair8", [NP, 128, 256], f8,
                            kind="ExternalInput").ap()
    ws8_ap = nc.dram_tensor("wblk8", [N_BLOCKS, 128, 128], f8,
                            kind="ExternalInput").ap()
    bias_ap = nc.dram_tensor("bias", [4, 128], f32,
                             kind="ExternalInput").ap()
    fcw_ap = nc.dram_tensor("fcw", [3, 128, NN], bf16,
                            kind="ExternalInput").ap()
    fcb_ap = nc.dram_tensor("fcb", [NN, 1], f32, kind="ExternalInput").ap()

    blk = _blk
    assert NG % blk == 0 and blk % gsub == 0

    with tile.TileContext(nc) as tc, ExitStack() as ctx:
        cpool = ctx.enter_context(tc.tile_pool(name="consts", bufs=1))
        p_xs = ctx.enter_context(tc.tile_pool(name="xs", bufs=bufs_xs))
        p_h = ctx.enter_context(tc.tile_pool(name="h", bufs=bufs_h))
        p_r = ctx.enter_context(tc.tile_pool(name="r", bufs=bufs_r))
        p_sig = ctx.enter_context(tc.tile_pool(name="sig", bufs=2))
        p_ph = ctx.enter_context(tc.tile_pool(name="ph", bufs=bufs_ph,
                                              space="PSUM"))
        p_pot = ctx.enter_context(tc.tile_pool(name="pot", bufs=bufs_pot,
                                               space="PSUM"))

        btile = cpool.tile([128, 4], f32, tag="bias")
        nc.sync.dma_start(btile[:], bias_ap.rearrange("l p -> p l"))
        bt = [btile[:, layer:layer + 1] for layer in range(4)]
        ftile = cpool.tile([128, 3 * NN], bf16, tag="fcw")
        nc.sync.dma_start(ftile[:].rearrange("p (i n) -> p i n", i=3),
                          fcw_ap.rearrange("i p n -> p i n"))
        fct = [ftile[:, i * NN:(i + 1) * NN] for i in range(3)]
        fcbt = cpool.tile([NN, 1], f32, tag="fcb")
        nc.sync.dma_start(fcbt[:], fcb_ap[:])

        def load_x(g):
            x8 = p_xs.tile([128, 3 * G], f8, tag="xs8")
            xst = p_xs.tile([128, 3 * G], bf16, tag="xs")
            if "tinyx" in cfg.get("probe", ""):
                # Timing probe: per-slice 16-col DMAs keep the dependency
                # edges but ~zero the DMA volume (wrong numerics).
                for j in range(3):
                    o = j * G
                    nc.sync.dma_start(x8[:, o:o + 16], xt8_ap[g][:, o:o + 16])
                    nc.sync.dma_start(xst[:, o:o + 16], xt_ap[g][:, o:o + 16])
            else:
                nc.sync.dma_start(x8[:], xt8_ap[g])
                nc.sync.dma_start(xst[:], xt_ap[g])
            return xst, x8

        xb_pre = ({g: load_x(g) for g in range(min(2, NG))}
                  if repeats == 1 else {})

        wp8 = cpool.tile([128, NP * 256], f8, tag="wpair8")
        nc.sync.dma_start(wp8[:].rearrange("p (k f) -> p k f", k=NP),
                          wp8_ap.rearrange("k p f -> p k f"))
        ws8 = cpool.tile([128, N_BLOCKS * 128], f8, tag="wblk8")
        nc.sync.dma_start(ws8[:].rearrange("p (k f) -> p k f", k=N_BLOCKS),
                          ws8_ap.rearrange("k p f -> p k f"))

        nodr = cfg.get("nodr", False)

        def emit_mm(ps, op, h3, start, stop, b0):
            """One matmul sub-slice: ps is the [128, 512] PSUM target AP,
            b0 the batch-column offset into the [128, t, G] h3 view."""
            bsl = slice(b0, b0 + GROUP)
            kind = op[0]
            if kind == "dr" and nodr:
                _, j0, step, pi = op
                ka, kb = EXEC_PAIRS[pi]
                nc.tensor.matmul(ps, lhsT=ws8[:, ka * 128:(ka + 1) * 128],
                                 rhs=h3[:, j0, bsl], start=start, stop=False)
                nc.tensor.matmul(ps, lhsT=ws8[:, kb * 128:(kb + 1) * 128],
                                 rhs=h3[:, j0 + step, bsl], start=False,
                                 stop=stop)
                return
            if kind == "dr":
                _, j0, step, pi = op
                rhs = (h3[:, j0:j0 + 2 * step - 1:step, bsl] if step == 2
                       else h3[:, j0:j0 + 2, bsl])
                nc.tensor.matmul(
                    ps,
                    lhsT=wp8[:, pi * 256:(pi + 1) * 256]
                        .rearrange("p (t m) -> p t m", t=2),
                    rhs=rhs, start=start, stop=stop, perf_mode=DR)
            else:
                _, j, k = op
                nc.tensor.matmul(
                    ps, lhsT=ws8[:, k * 128:(k + 1) * 128],
                    rhs=h3[:, j, bsl], start=start, stop=stop)

        probe = cfg.get("probe", "")

        def relu_move(e, out, ps, layer):
            if "tinyrelu" in probe:
                # Timing probe: keep dataflow edges, shrink the work (wrong
                # numerics) to expose the PE+DMA+schedule envelope.
                nc.scalar.activation(out[:, :16], ps[:, :16], Relu,
                                     bias=bt[layer])
                return
            if e == "A":
                nc.scalar.activation(out, ps[:], Relu, bias=bt[layer])
            else:
                nc.vector.tensor_scalar(out=out, in0=ps[:],
                                        scalar1=bt[layer], scalar2=0.0,
                                        op0=mybir.AluOpType.add,
                                        op1=mybir.AluOpType.max)

        def mk_st(g):
            pre = xb_pre.pop(g, None)
            xst, x8 = pre if pre is not None else load_x(g)
            return {
                "xs": [xst[:, j * G:(j + 1) * G] for j in range(3)],
                "h3": x8[:].rearrange("p (t b) -> p t b", t=3),
                "h": None,
            }

        h3b = cfg.get("h3b", 7 if wave else bufs_h)

        def layer_step(ssts, layer):
            """One GCN layer for the groups in ssts, matmuls interleaved."""
            for st in ssts:
                if layer < 3:
                    ht3 = p_h.tile([128, 3 * G], f8, tag=f"hp{layer}")
                    st["houts"] = [ht3[:, i * G:(i + 1) * G]
                                   for i in range(3)]
                    st["h3n"] = ht3[:].rearrange("p (t b) -> p t b", t=3)
                else:
                    st["houts"] = [
                        p_h.tile([128, G], bf16, tag=f"h{i}",
                                 name=f"h{i}", bufs=h3b)[:]
                        for i in range(3)]
                    st["h3n"] = None
            for i in range(3):
                pss = [p_ph.tile([128, G], f32, tag="ph", name="ps")
                       for _ in ssts]
                ops = EXEC_OPS[(layer, i)]
                for hb in range(NSUB):
                    b0 = hb * GROUP
                    for bi, op in enumerate(ops):
                        for st, ps in zip(ssts, pss):
                            emit_mm(ps[:, b0:b0 + GROUP], op, st["h3"],
                                    start=(bi == 0),
                                    stop=(bi == len(ops) - 1), b0=b0)
                for st, ps in zip(ssts, pss):
                    relu_move(rtab[layer][i], st["houts"][i], ps, layer)
            for st in ssts:
                st["h3"] = st["h3n"]
                st["h"] = st["houts"]

        def tail_step(g0, csts):
            """Residual + fc heads + sigmoid + store for len(csts) groups."""
            nsig = len(csts)
            psfc = p_pot.tile([NN, nsig * G], f32, tag="ot", name="psfc")
            halves = [(s, s * G + hb * GROUP) for s in range(nsig)
                      for hb in range(NSUB)]

            def fc_mm(s, o, i, rhs_full, start, stop):
                b0 = o - s * G
                nc.tensor.matmul(psfc[:, o:o + GROUP], lhsT=fct[i],
                                 rhs=rhs_full[:, b0:b0 + GROUP],
                                 start=start, stop=stop)

            if fcres:
                for i in range(3):
                    for s, o in halves:
                        fc_mm(s, o, i, csts[s]["h"][i], i == 0, False)
                    for s, o in halves:
                        fc_mm(s, o, i, csts[s]["xs"][i], False, i == 2)
            else:
                for s, st in enumerate(csts):
                    rs = []
                    for i in range(3):
                        ri = p_r.tile([128, G], bf16, tag=f"r{i}")
                        eng = nc.vector if addeng == "V" else nc.gpsimd
                        if "tinyadd" in probe:
                            eng.tensor_add(out=ri[:, :16],
                                           in0=st["h"][i][:, :16],
                                           in1=st["xs"][i][:, :16])
                        else:
                            eng.tensor_add(out=ri[:], in0=st["h"][i],
                                           in1=st["xs"][i])
                        rs.append(ri)
                    for hb in range(NSUB):
                        o = s * G + hb * GROUP
                        for i in range(3):
                            fc_mm(s, o, i, rs[i][:], i == 0, i == 2)
            sig = p_sig.tile([NN, nsig * G], f32, tag="sig")
            nc.scalar.activation(sig[:], psfc[:], Sigmoid, bias=fcbt[:])
            # Optional: y stores on the Pool engine's DMA queue (measured
            # slower than the shared SP queue — keep off).
            dq_store = nc.gpsimd if cfg.get("dmaq", False) else nc.sync
            dq_store.dma_start(
                y_ap[g0:g0 + nsig].rearrange("g n b -> n g b"),
                sig[:].rearrange("n (g b) -> n g b", g=nsig))

        def blk_body(g0):
            sts = [mk_st(g) for g in range(g0, g0 + blk)]
            for s0 in range(0, blk, gsub):
                for layer in range(4):
                    layer_step(sts[s0:s0 + gsub], layer)
            for c0 in range(0, blk, sigb):
                tail_step(g0 + c0, sts[c0:c0 + sigb])

        def run_groups():
            if not cfg.get("wave"):
                for g0 in range(0, NG, blk):
                    blk_body(g0)
                return
            # Software-pipelined wavefront over pairs of groups: in wave w,
            # pair w loads its x, pair w-1 runs layer 0, ... pair w-4 runs
            # layer 3, pair w-5 runs the tail.  Every instruction in a wave
            # depends only on results from previous waves, so each engine's
            # strict-FIFO queue always has ready work at its head.
            W = gsub
            P = NG // W
            states = {}
            for w in range(P + 6):
                p = w - 5
                if 0 <= p < P:
                    tail_step(p * W, states.pop(p))
                for layer in (3, 2, 1, 0):
                    p = w - 1 - layer
                    if 0 <= p < P:
                        layer_step(states[p], layer)
                if w < P:
                    states[w] = [mk_st(w * W + k) for k in range(W)]

        if repeats == 1:
            run_groups()
        else:
            hint = (mybir.EngineType.PE, mybir.EngineType.Activation,
                    mybir.EngineType.DVE, mybir.EngineType.SP,
                    mybir.EngineType.Pool)
            with tc.For_i(0, repeats, hint_engines=hint):
                run_groups()

    nc.compile()
    return nc


class Runner:
    """Compiled program + cached jitted PJRT executable over the 8 cores.

    Mirrors concourse.bass2jax.run_bass_via_pjrt, but keeps the jitted
    callable and accepts device-resident inputs so repeated timed calls do
    not re-trace or re-transfer."""

    def __init__(self, nc):
        import jax
        import numpy as _np
        from jax.sharding import Mesh, PartitionSpec, NamedSharding
        from jax.experimental.shard_map import shard_map
        import concourse.mybir as mybir
        from concourse import bass2jax

        bass2jax.install_neuronx_cc_hook()
        self.nc = nc
        assert nc.dbg_addr is None
        partition_name = (nc.partition_id_tensor.name
                          if nc.partition_id_tensor else None)

        in_names, out_names, out_avals, zero_outs = [], [], [], []
        for alloc in nc.m.functions[0].allocations:
            if not isinstance(alloc, mybir.MemoryLocationSet):
                continue
            name = alloc.memorylocations[0].name
            if alloc.kind == "ExternalInput":
                if name == partition_name:
                    continue
                in_names.append(name)
            elif alloc.kind == "ExternalOutput":
                shape = tuple(alloc.tensor_shape)
                dtype = mybir.dt.np(alloc.dtype)
                out_names.append(name)
                out_avals.append(jax.core.ShapedArray(shape, dtype))
                zero_outs.append(_np.zeros(shape, dtype))
        self.in_names = list(in_names)
        self.out_names = out_names
        self.out_avals = out_avals
        self.zero_outs = zero_outs
        n_params = len(in_names)
        n_outs = len(out_avals)
        all_in_names = in_names + out_names
        if partition_name is not None:
            all_in_names = all_in_names + [partition_name]

        def _body(*args):
            operands = list(args)
            if partition_name is not None:
                operands.append(bass2jax.partition_id_tensor())
            outs = bass2jax._bass_exec_p.bind(
                *operands,
                out_avals=tuple(out_avals),
                in_names=tuple(all_in_names),
                out_names=tuple(out_names),
                lowering_input_output_aliases=(),
                sim_require_finite=True,
                sim_require_nnan=True,
                nc=nc,
            )
            return tuple(outs)

        devices = jax.devices()[:N_CORES]
        self.mesh = Mesh(_np.asarray(devices), ("core",))
        self.sharding = NamedSharding(self.mesh, PartitionSpec("core"))
        in_specs = (PartitionSpec("core"),) * (n_params + n_outs)
        out_specs = (PartitionSpec("core"),) * n_outs
        self.jitted = jax.jit(
            shard_map(_body, mesh=self.mesh, in_specs=in_specs,
                      out_specs=out_specs, check_rep=False),
            keep_unused=True,
        )
        self._jax = jax

    def put_inputs(self, in_maps):
        """in_maps: list of N_CORES dicts name->np.  Returns device arrays."""
        import numpy as _np
        concat = [
            _np.concatenate([_np.asarray(m[name]) for m in in_maps], axis=0)
            for name in self.in_names
        ]
        dev = [self._jax.device_put(a, self.sharding) for a in concat]
        # The zero "output operands" are never read by the NEFF (no
        # input/output aliasing is declared); upload them once and reuse.
        self._zeros_dev = [
            self._jax.device_put(
                self._jax.numpy.zeros((N_CORES * z.shape[0], *z.shape[1:]),
                                      z.dtype),
                self.sharding)
            for z in self.zero_outs
        ]
        return dev

    def run(self, dev_inputs):
        outs = self.jitted(*dev_inputs, *self._zeros_dev)
        outs = [self._jax.block_until_ready(o) for o in outs]
        return {
            name: outs[i]
            for i, name in enumerate(self.out_names)
        }


_RUNNERS = {}


def get_runner(repeats: int = 1, cfg: dict | None = None) -> Runner:
    key = (repeats, tuple(sorted((cfg or {}).items())))
    if key not in _RUNNERS:
        build = build_program_v2 if (cfg or {}).get("v2") else build_program
        _RUNNERS[key] = Runner(build(repeats, cfg))
    return _RUNNERS[key]


def _make_in_maps(inputs, wdtype="f32r", xdtype=None):
    if xdtype is None:
        xdtype = wdtype if wdtype != "f32r" else "f32r"
    x = np.ascontiguousarray(np.asarray(inputs["x"], np.float32))
    assert x.shape == (BATCH, NN, FEAT)
    consts = build_consts(
        W=[np.asarray(inputs[f"W{i+1}"], np.float32) for i in range(4)],
        b=[np.asarray(inputs[f"b{i+1}"], np.float32) for i in range(4)],
        fc_w=np.asarray(inputs["fc_w"], np.float32),
        fc_b=np.asarray(inputs["fc_b"], np.float32),
    )
    if wdtype == "bf16":
        import ml_dtypes
        consts["wblk"] = consts["wblk"].astype(ml_dtypes.bfloat16)
        consts["fcw"] = consts["fcw"].astype(ml_dtypes.bfloat16)
    if xdtype == "bf16":
        import ml_dtypes
        x = x.astype(ml_dtypes.bfloat16)
        consts["eye128"] = consts["eye128"].astype(ml_dtypes.bfloat16)
    x_sh = x.reshape(N_CORES, PER_CORE, NN * FEAT)
    return [{"x": x_sh[c], **consts} for c in range(N_CORES)]


FP8_SETS = {None: frozenset(), "l234": frozenset({1, 2, 3}),
            "all": frozenset({0, 1, 2, 3})}


def _maps_for_v2(inputs, G=GROUP):
    """Input maps for build_program_v2: host-pretransposed x in bf16 + fp8,
    _exec_plan weight tensors, fc/bias consts."""
    import ml_dtypes
    NG = PER_CORE // G
    consts = build_consts(
        W=[np.asarray(inputs[f"W{i+1}"], np.float32) for i in range(4)],
        b=[np.asarray(inputs[f"b{i+1}"], np.float32) for i in range(4)],
        fc_w=np.asarray(inputs["fc_w"], np.float32),
        fc_b=np.asarray(inputs["fc_b"], np.float32),
    )
    wpair = np.zeros((len(EXEC_PAIRS), 128, 256), np.float32)
    for pi, (ka, kb) in enumerate(EXEC_PAIRS):
        wpair[pi, :, :128] = consts["wblk"][ka]
        wpair[pi, :, 128:] = consts["wblk"][kb]
    f8 = ml_dtypes.float8_e4m3
    com = {
        "wpair8": wpair.astype(f8),
        "wblk8": consts["wblk"].astype(f8),
        "bias": consts["bias"],
        "fcw": consts["fcw"].astype(ml_dtypes.bfloat16),
        "fcb": consts["fcb"],
    }
    x = np.ascontiguousarray(np.asarray(inputs["x"], np.float32))
    xt = np.ascontiguousarray(
        x.reshape(N_CORES, NG, G, 3, 128)
        .transpose(0, 1, 4, 3, 2)).reshape(N_CORES, NG, 128, 3 * G)
    xt_bf = xt.astype(ml_dtypes.bfloat16)
    xt_f8 = xt.astype(f8)
    return [{"xt": xt_bf[c], "xt8": xt_f8[c], **com} for c in range(N_CORES)]


def _maps_for_cfg(inputs, cfg):
    cfg = dict(cfg or {})
    if cfg.get("v2"):
        return _maps_for_v2(inputs, G=cfg.get("G", GROUP))
    xdt = cfg.get("xdtype", "f32r")
    wdt = cfg.get("wdtype", xdt)
    maps = _make_in_maps(inputs, wdtype=wdt, xdtype=xdt)
    fp8set = FP8_SETS[cfg.get("fp8")]
    if fp8set:
        import ml_dtypes
        consts = build_consts(
            W=[np.asarray(inputs[f"W{i+1}"], np.float32) for i in range(4)],
            b=[np.asarray(inputs[f"b{i+1}"], np.float32) for i in range(4)],
            fc_w=np.asarray(inputs["fc_w"], np.float32),
            fc_b=np.asarray(inputs["fc_b"], np.float32),
        )
        pairs = _pair_plan(fp8set)
        wpair = np.zeros((len(pairs), 128, 256), np.float32)
        for pi, (_l, _i, _j0, (ka, kb)) in enumerate(pairs):
            if ka is not None:
                wpair[pi, :, :128] = consts["wblk"][ka]
            if kb is not None:
                wpair[pi, :, 128:] = consts["wblk"][kb]
        wp8 = wpair.astype(ml_dtypes.float8_e4m3)
        for m in maps:
            m["wpair8"] = wp8
    if cfg.get("hostt"):
        import ml_dtypes
        x = np.ascontiguousarray(np.asarray(inputs["x"], np.float32))
        # [C, NG, 512, 3, 128] -> feature-major [C, NG, 128(p), 3(j), 512(b)]
        xt = np.ascontiguousarray(
            x.reshape(N_CORES, N_GROUPS, GROUP, 3, 128)
            .transpose(0, 1, 4, 3, 2)).reshape(N_CORES, N_GROUPS, 128,
                                               3 * GROUP)
        xt_bf = xt.astype(ml_dtypes.bfloat16)
        xt_f8 = (xt.astype(ml_dtypes.float8_e4m3)
                 if 0 in fp8set else None)
        for c, m in enumerate(maps):
            m.pop("x", None)
            m["xt"] = xt_bf[c]
            if xt_f8 is not None:
                m["xt8"] = xt_f8[c]
    return maps


def unpack_y(y_raw: np.ndarray, G: int = GROUP) -> np.ndarray:
    """Device output [N_CORES * NG, NN, G] -> [BATCH, NN]."""
    NG = PER_CORE // G
    y = y_raw.reshape(N_CORES, NG, NN, G)
    return np.ascontiguousarray(
        y.transpose(0, 1, 3, 2).reshape(BATCH, NN))


# Best measured configuration (see NOTES.md): all-fp8 exec plan on 1024-wide
# groups, 6-stage software-pipelined wave schedule.
DEFAULT_CFG = {"v2": True, "G": 1024, "wave": True}


def kernel(**inputs) -> np.ndarray:
    runner = get_runner(repeats=1, cfg=DEFAULT_CFG)
    dev = runner.put_inputs(_maps_for_cfg(inputs, DEFAULT_CFG))
    out = runner.run(dev)
    return unpack_y(np.asarray(out["y"]), G=DEFAULT_CFG.get("G", GROUP))



# revision 46
# speedup vs baseline: 1.5534x; 1.0002x over previous
"""Trainium2 Bass kernel for the 6-node GCN classification model.

Math: each GCN layer is h' = relu(A @ h @ W^T + b) on [B, 6, 64], where A is
the 6x6 normalized adjacency (with self loops; fill=1.0 for layers 1-2,
fill=2.0 for layers 3-4).  With the 6 nodes stacked in pairs on the 128 SBUF
partitions (2 nodes x 64 features), the fused per-layer operator A (x) W is a
384x384 block matrix; each nonzero 128x128 block becomes one TensorE matmul
accumulated in PSUM.  Node pairings alternate between two stackings chosen so
the total block count over the 4 layers is the provable minimum (26).

Pipeline per 512-batch group, fully fused in SBUF (x is read from HBM exactly
once, only the [B, 6] sigmoid output is written back):
  DMA x (batch-major, contiguous) -> PE transpose to feature-major stacked
  -> 4 x (block matmuls f32r -> ACT bias+ReLU) -> DVE residual add
  -> fc-head matmuls -> ACT sigmoid -> PE transpose back -> DMA out.

Sharding: pure data parallel over the batch dim across the 8 NeuronCores.
"""

import math
from contextlib import ExitStack, nullcontext as _nullctx

import numpy as np

N_CORES = 8
BATCH = 131072
PER_CORE = BATCH // N_CORES  # 16384
NN = 6
FEAT = 64
GROUP = 512
N_GROUPS = PER_CORE // GROUP  # 32

SRC = [1, 2, 0, 2, 1, 3, 2, 4, 3, 5, 3, 4]
DST = [0, 0, 1, 1, 2, 2, 3, 3, 4, 4, 5, 5]

# Node pair stackings per layer boundary (chain start == chain end so the
# residual/fc read the same stacking the input transposes produce).
S_A = [(0, 1), (2, 3), (4, 5)]
S_B = [(0, 5), (1, 2), (3, 4)]
CHAIN = [S_A, S_B, S_A, S_B, S_A]  # layer l maps CHAIN[l] -> CHAIN[l+1]
_IPERMS = [(0, 1, 2), (0, 2, 1), (1, 0, 2), (1, 2, 0), (2, 0, 1), (2, 1, 0)]

# Engine assignments for the post-matmul elementwise ops.  GPSIMD (Pool)
# cannot access PSUM on TRN2, so the PSUM->SBUF relu moves and xt copies can
# only go to ACT (A) / DVE (V); Pool (P) can take SBUF->SBUF residual adds.
ENG_TABLES = {
    # baseline: ACT does everything except one relu per early layer on DVE
    "base": dict(
        relu=[("A", "A", "V"), ("A", "A", "V"), ("A", "A", "V"),
              ("A", "A", "A")],
        cp=("V", "V", "V"), add=("V", "V", "V")),
    # even ACT/DVE split, residual adds on Pool
    "split": dict(
        relu=[("A", "V", "A"), ("V", "A", "V"), ("A", "V", "A"),
              ("V", "A", "V")],
        cp=("V", "A", "V"), add=("P", "P", "P")),
    # even ACT/DVE split, residual adds on DVE
    "splitv": dict(
        relu=[("A", "V", "A"), ("V", "A", "V"), ("A", "V", "A"),
              ("V", "A", "V")],
        cp=("V", "A", "V"), add=("V", "V", "V")),
    # fp8+dmat balance: ACT carries 5 relus + sigmoid, DVE 7 relus, Pool adds
    "split8": dict(
        relu=[("A", "V", "V"), ("A", "V", "A"), ("V", "A", "V"),
              ("A", "V", "V")],
        cp=("V", "A", "V"), add=("P", "P", "P")),
}


def _gcn_A(fill: float) -> np.ndarray:
    """Dense [6, 6] aggregation matrix A[dst, src] incl. weighted self loops."""
    src = SRC + list(range(NN))
    dst = DST + list(range(NN))
    w = [1.0] * len(SRC) + [fill] * NN
    deg = np.zeros(NN, np.float64)
    for s, d, ww in zip(src, dst, w):
        deg[d] += ww
    dinv = np.where(deg > 0, 1.0 / np.sqrt(deg), 0.0)
    A = np.zeros((NN, NN), np.float64)
    for s, d, ww in zip(src, dst, w):
        A[d, s] += dinv[s] * ww * dinv[d]
    return A


def _block_plan():
    """Static plan: for each layer, the nonzero (out_tile, in_tile) blocks.

    Returns [layer][out_tile] -> list of in_tile indices, using the support of
    A (same for both fill values)."""
    S = np.zeros((NN, NN), bool)
    for s, d in zip(SRC, DST):
        S[d, s] = True
    for i in range(NN):
        S[i, i] = True
    plan = []
    for layer in range(4):
        inp, outp = CHAIN[layer], CHAIN[layer + 1]
        lplan = []
        for (n0, n1) in outp:
            js = []
            for j, (m0, m1) in enumerate(inp):
                if S[n0, m0] or S[n0, m1] or S[n1, m0] or S[n1, m1]:
                    js.append(j)
            lplan.append(js)
        plan.append(lplan)
    return plan


BLOCK_PLAN = _block_plan()
N_BLOCKS = sum(len(js) for lp in BLOCK_PLAN for js in lp)  # 26


def _pair_plan(fp8set):
    """DoubleRow pair schedule for the fp8 layers.

    Each entry: (layer, out_tile, rhs_j0, (blkA|None, blkB|None)) meaning one
    DoubleRow matmul with rhs = h[j0], h[j0+1] and the two stationary K-tiles
    holding A-scaled weight blocks (None = zero block).  blkX indexes into
    the flat wblk ordering used by build_consts."""
    woff = [0]
    for layer in range(4):
        for i in range(3):
            woff.append(woff[-1] + len(BLOCK_PLAN[layer][i]))
    plan = []
    for layer in sorted(fp8set):
        for i in range(3):
            js = BLOCK_PLAN[layer][i]
            k0 = woff[layer * 3 + i]
            ks = {j: k0 + bi for bi, j in enumerate(js)}
            if js == [0, 1] or js == [1, 2]:
                plan.append((layer, i, js[0], (ks[js[0]], ks[js[1]])))
            elif js == [0, 1, 2]:
                plan.append((layer, i, 0, (ks[0], ks[1])))
                plan.append((layer, i, 1, (None, ks[2])))
            elif js == [0, 2]:
                plan.append((layer, i, 0, (ks[0], None)))
                plan.append((layer, i, 1, (None, ks[2])))
            else:
                raise AssertionError(js)
    return plan


def _exec_plan():
    """Improved fp8 matmul plan (all 4 layers fp8).

    Returns (ops, pairs): ops[(layer, i)] = list of
    ('dr', j0, step, pair_idx) | ('mm', j, blk_idx).  A 'dr' is one DoubleRow
    matmul with rhs slots (j0, j0+step) and stationary pair pair_idx in the
    wpair8 tensor; an 'mm' is a plain fp8 matmul on flat block blk_idx of
    wblk8.  Versus the legacy _pair_plan this never issues a half-empty
    DoubleRow: [0,2] inputs use a stride-2 rhs AP and [0,1,2] runs as one
    DoubleRow plus one cheap normal matmul.
    """
    woff = [0]
    for layer in range(4):
        for i in range(3):
            woff.append(woff[-1] + len(BLOCK_PLAN[layer][i]))
    ops = {}
    pairs = []
    for layer in range(4):
        for i in range(3):
            js = BLOCK_PLAN[layer][i]
            k0 = woff[layer * 3 + i]
            ks = {j: k0 + bi for bi, j in enumerate(js)}
            lops = []
            if js == [0, 1] or js == [1, 2]:
                lops.append(("dr", js[0], 1, len(pairs)))
                pairs.append((ks[js[0]], ks[js[1]]))
            elif js == [0, 2]:
                lops.append(("dr", 0, 2, len(pairs)))
                pairs.append((ks[0], ks[2]))
            elif js == [0, 1, 2]:
                lops.append(("dr", 0, 1, len(pairs)))
                pairs.append((ks[0], ks[1]))
                lops.append(("mm", 2, ks[2]))
            else:
                raise AssertionError(js)
            ops[(layer, i)] = lops
    return ops, pairs


EXEC_OPS, EXEC_PAIRS = _exec_plan()


def build_consts(W, b, fc_w, fc_b):
    """Host-side constant tensors fed to the device as DRAM inputs.

    W: list of 4 [64, 64] arrays; b: list of 4 [64]; fc_w [6, 64]; fc_b [6].
    """
    A = [_gcn_A(1.0), _gcn_A(1.0), _gcn_A(2.0), _gcn_A(2.0)]
    wblk = np.zeros((N_BLOCKS, 128, 128), np.float32)
    k = 0
    for layer in range(4):
        inp, outp = CHAIN[layer], CHAIN[layer + 1]
        Wt = W[layer].T.astype(np.float64)  # [f, g] = W[g, f]
        for i, (n0, n1) in enumerate(outp):
            for j in BLOCK_PLAN[layer][i]:
                m0, m1 = inp[j]
                blk = np.zeros((128, 128), np.float64)
                for dj, m in enumerate((m0, m1)):
                    for do, n in enumerate((n0, n1)):
                        a = A[layer][n, m]
                        if a != 0.0:
                            blk[dj * 64:(dj + 1) * 64, do * 64:(do + 1) * 64] = a * Wt
                wblk[k] = blk.astype(np.float32)
                k += 1
    assert k == N_BLOCKS

    bias = np.zeros((4, 128), np.float32)
    for layer in range(4):
        bias[layer] = np.tile(b[layer], 2)

    fcw = np.zeros((3, 128, NN), np.float32)
    for i, (n0, n1) in enumerate(CHAIN[4]):
        for do, n in enumerate((n0, n1)):
            fcw[i, do * 64:(do + 1) * 64, n] = fc_w[n]

    return {
        "wblk": wblk,
        "bias": bias,
        "fcw": fcw,
        "fcb": fc_b.astype(np.float32).reshape(NN, 1),
        "eye128": np.eye(128, dtype=np.float32),
    }


def build_program(repeats: int = 1, cfg: dict | None = None):
    """Build + schedule + compile the Bass/Tile program. Returns nc."""
    import concourse.tile as tile
    import concourse.mybir as mybir
    from concourse import bacc

    cfg = dict(cfg or {})
    bufs_xb = cfg.get("xb", 3)
    bufs_xs = cfg.get("xs", 3)
    bufs_h = cfg.get("h", 4)
    bufs_r = cfg.get("r", 2)
    bufs_px = cfg.get("px", 2)
    bufs_ph = cfg.get("ph", 5)
    bufs_pfc = cfg.get("pfc", 1)
    bufs_pot = cfg.get("pot", 1)
    xt_in_ph = cfg.get("xt_in_ph", False)
    ot_in_pfc = cfg.get("ot_in_pfc", False)
    xdt_name = cfg.get("xdtype", "f32r")
    wdt_name = cfg.get("wdtype", xdt_name)
    dmat = cfg.get("dmat", False)  # XBAR DMA-transpose input path (bf16 only)
    hostt = cfg.get("hostt", False)  # host pre-transposed x upload
    if dmat or hostt:
        assert xdt_name == "bf16"
        bufs_ph = cfg.get("ph", 7)  # px pool unused -> spend its PSUM banks
    if cfg.get("bigps"):
        bufs_ph = cfg.get("ph", 2)  # [128, 3*GROUP] tiles = 3 banks each
    fp8set = FP8_SETS[cfg.get("fp8")]
    pairs = _pair_plan(fp8set) if fp8set else []
    ENG = ENG_TABLES[cfg.get("eng", "base")]
    RELU_ENG, CP_ENG, ADD_ENG = ENG["relu"], ENG["cp"], ENG["add"]

    f32 = mybir.dt.float32
    f32r = mybir.dt.float32r
    _DT = {"f32r": mybir.dt.float32r, "bf16": mybir.dt.bfloat16}
    wdt = _DT[wdt_name]
    adt = _DT[xdt_name]  # on-chip dtype of x and of the layer activations
    Relu = mybir.ActivationFunctionType.Relu
    Sigmoid = mybir.ActivationFunctionType.Sigmoid
    Copy = mybir.ActivationFunctionType.Copy
    f8 = mybir.dt.float8e4
    DR = mybir.MatmulPerfMode.DoubleRow

    nc = bacc.Bacc("TRN2", target_bir_lowering=False, debug=False,
                   num_devices=N_CORES)

    if hostt:
        xt_ap = nc.dram_tensor("xt", [N_GROUPS, 128, 3 * GROUP], adt,
                               kind="ExternalInput").ap()
        if 0 in fp8set:
            xt8_ap = nc.dram_tensor("xt8", [N_GROUPS, 128, 3 * GROUP],
                                    mybir.dt.float8e4,
                                    kind="ExternalInput").ap()
    else:
        x_ap = nc.dram_tensor("x", [PER_CORE, NN * FEAT], adt,
                              kind="ExternalInput").ap()
    y_ap = nc.dram_tensor("y", [N_GROUPS, NN, GROUP], f32,
                          kind="ExternalOutput").ap()
    wblk_ap = nc.dram_tensor("wblk", [N_BLOCKS, 128, 128], wdt,
                             kind="ExternalInput").ap()
    bias_ap = nc.dram_tensor("bias", [4, 128], f32,
                             kind="ExternalInput").ap()
    fcw_ap = nc.dram_tensor("fcw", [3, 128, NN], wdt,
                            kind="ExternalInput").ap()
    fcb_ap = nc.dram_tensor("fcb", [NN, 1], f32, kind="ExternalInput").ap()
    eye128_ap = nc.dram_tensor("eye128", [128, 128], adt,
                               kind="ExternalInput").ap()
    if pairs:
        wp8_ap = nc.dram_tensor("wpair8", [len(pairs), 128, 256], f8,
                                kind="ExternalInput").ap()

    SB = GROUP // 128  # 4 batch sub-tiles per group

    with tile.TileContext(nc) as tc, ExitStack() as ctx:
        cpool = ctx.enter_context(tc.tile_pool(name="consts", bufs=1))
        p_xb = ctx.enter_context(tc.tile_pool(name="xb", bufs=bufs_xb))
        p_xs = ctx.enter_context(tc.tile_pool(name="xs", bufs=bufs_xs))
        p_h = ctx.enter_context(tc.tile_pool(name="h", bufs=bufs_h))
        p_r = ctx.enter_context(tc.tile_pool(name="r", bufs=bufs_r))
        p_sig = ctx.enter_context(tc.tile_pool(name="sig", bufs=2))
        p_ob = ctx.enter_context(tc.tile_pool(name="ob", bufs=2))
        p_ph = ctx.enter_context(tc.tile_pool(name="ph", bufs=bufs_ph, space="PSUM"))
        p_px = p_ph if xt_in_ph else ctx.enter_context(
            tc.tile_pool(name="px", bufs=bufs_px, space="PSUM"))
        p_pot = ctx.enter_context(
            tc.tile_pool(name="pot", bufs=bufs_pot, space="PSUM"))
        eye128 = cpool.tile([128, 128], adt, tag="eye128")
        nc.sync.dma_start(eye128[:], eye128_ap[:])
        btile = cpool.tile([128, 4], f32, tag="bias")
        nc.sync.dma_start(btile[:], bias_ap.rearrange("l p -> p l"))
        bt = [btile[:, layer:layer + 1] for layer in range(4)]
        ftile = cpool.tile([128, 3 * NN], wdt, tag="fcw")
        nc.sync.dma_start(ftile[:].rearrange("p (i n) -> p i n", i=3),
                          fcw_ap.rearrange("i p n -> p i n"))
        fct = [ftile[:, i * NN:(i + 1) * NN] for i in range(3)]
        fcbt = cpool.tile([NN, 1], f32, tag="fcb")
        nc.sync.dma_start(fcbt[:], fcb_ap[:])
        def load_xb(g):
            xb = p_xb.tile([128, SB * NN * FEAT], adt, tag="xb")
            nc.sync.dma_start(
                xb[:].rearrange("p (s f) -> p s f", s=SB),
                x_ap[g * GROUP:(g + 1) * GROUP, :].rearrange(
                    "(s p) f -> p s f", p=128),
            )
            return xb

        def load_xs_t(g):
            # XBAR DMA transpose: [512, 384] bf16 rows -> feature-major
            # [128, 3, 512] (chunk j holds features 128j..128j+127, which is
            # exactly the S_A node-pair stacking).  Replaces 12 PE transposes
            # and 3 PSUM->SBUF copies per group.
            xst = p_xs.tile([128, 3 * GROUP], adt, tag="xs")
            nc.sync.dma_start_transpose(
                xst[:].rearrange("p (j b) -> p j b", j=3),
                x_ap[g * GROUP:(g + 1) * GROUP, :])
            return xst

        def load_xs_host(g):
            # Host pre-transposed x: plain contiguous DMA per group (bf16 for
            # layer-1/residual, plus fp8 copy when layer 1 runs DoubleRow).
            xst = p_xs.tile([128, 3 * GROUP], adt, tag="xs")
            nc.sync.dma_start(xst[:], xt_ap[g])
            x8 = None
            if 0 in fp8set:
                x8 = p_xs.tile([128, 3 * GROUP], f8, tag="xs8")
                nc.sync.dma_start(x8[:], xt8_ap[g])
            return (xst, x8)

        # The first groups' x tiles go before the big weight DMAs so the
        # transposes can start while the weights stream in.  (Single-pass
        # builds only: under a repeat loop these DMAs would not replay.)
        load_x_pre = (load_xs_host if hostt
                      else (load_xs_t if dmat else load_xb))
        xb_pre = ({g: load_x_pre(g) for g in range(min(2, N_GROUPS))}
                  if repeats == 1 else {})

        # Block weights in two DMAs (layer-0 blocks first so the first
        # group's matmuls can start before the rest of the weights land).
        nb0 = sum(len(js) for js in BLOCK_PLAN[0])
        wtile = cpool.tile([128, N_BLOCKS * 128], wdt, tag="wblk")
        nc.sync.dma_start(
            wtile[:, :nb0 * 128].rearrange("p (k f) -> p k f", k=nb0),
            wblk_ap[:nb0].rearrange("k p f -> p k f"))
        nc.sync.dma_start(
            wtile[:, nb0 * 128:].rearrange("p (k f) -> p k f", k=N_BLOCKS - nb0),
            wblk_ap[nb0:].rearrange("k p f -> p k f"))
        wt = [wtile[:, k * 128:(k + 1) * 128] for k in range(N_BLOCKS)]

        if pairs:
            wp8 = cpool.tile([128, len(pairs) * 256], f8, tag="wpair8")
            nc.sync.dma_start(
                wp8[:].rearrange("p (k f) -> p k f", k=len(pairs)),
                wp8_ap.rearrange("k p f -> p k f"))
            pair_by_li = {}
            for pi, (pl, i, j0, _ks) in enumerate(pairs):
                pair_by_li.setdefault((pl, i), []).append((pi, j0))


        probe = cfg.get("probe", "")

        def relu_move(e, ht, ps, layer):
            if "tinyrelu" in probe:
                # Probe mode: keep the dataflow edges but shrink the
                # elementwise work to ~nothing (timing only, wrong numerics).
                nc.scalar.activation(ht[:, :16], ps[:, :16], Relu,
                                     bias=bt[layer])
                return
            if cfg.get("relu2"):
                # Halve the relu latency on the critical path: ACT and DVE
                # each process half the tile concurrently.
                half = GROUP // 2
                nc.scalar.activation(ht[:, :half], ps[:, :half], Relu,
                                     bias=bt[layer])
                nc.vector.tensor_scalar(out=ht[:, half:], in0=ps[:, half:],
                                        scalar1=bt[layer], scalar2=0.0,
                                        op0=mybir.AluOpType.add,
                                        op1=mybir.AluOpType.max)
                return
            if e == "A":
                nc.scalar.activation(ht[:], ps[:], Relu, bias=bt[layer])
            else:
                eng = nc.vector if e == "V" else nc.gpsimd
                eng.tensor_scalar(out=ht[:], in0=ps[:], scalar1=bt[layer],
                                  scalar2=0.0, op0=mybir.AluOpType.add,
                                  op1=mybir.AluOpType.max)

        def group_body(g):
            first_t = None
            xs8t_loaded = None
            if hostt:
                pre = xb_pre.pop(g, None)
                xst, xs8t_loaded = pre if pre is not None else load_xs_host(g)
                xs = [xst[:, j * GROUP:(j + 1) * GROUP] for j in range(3)]
            elif dmat:
                xst = xb_pre.pop(g, None)
                if xst is None:
                    xst = load_xs_t(g)
                xs = [xst[:, j * GROUP:(j + 1) * GROUP] for j in range(3)]
            else:
                # Load [512, 384] rows batch-major: partition = batch % 128.
                xb = xb_pre.pop(g, None)
                if xb is None:
                    xb = load_xb(g)
                # Transpose to feature-major stacked (pairs = CHAIN[0]).
                xs = []
                xts = []
                for j in range(3):
                    xt = p_px.tile([128, GROUP], adt,
                                   tag="ph" if xt_in_ph else "xt")
                    for s in range(SB):
                        ti = nc.tensor.transpose(
                            xt[:, s * 128:(s + 1) * 128],
                            xb[:, s * NN * FEAT + j * 128:
                               s * NN * FEAT + (j + 1) * 128],
                            eye128[:],
                        )
                        if first_t is None:
                            first_t = ti
                    xts.append(xt)
                for j in range(3):
                    xsj = p_xs.tile([128, GROUP], adt, tag=f"xs{j}")
                    e = CP_ENG[j]
                    if e == "A":
                        nc.scalar.activation(xsj[:], xts[j][:], Copy)
                    else:
                        eng = nc.vector if e == "V" else nc.gpsimd
                        eng.tensor_copy(out=xsj[:], in_=xts[j][:])
                    xs.append(xsj)

            h = xs
            iperm = _IPERMS[cfg.get("iorder", 0)]
            woff = [0]
            for layer in range(4):
                for i in range(3):
                    woff.append(woff[-1] + len(BLOCK_PLAN[layer][i]))

            # h3: [p, t, b] single-tile view of the CURRENT h list, required
            # as the DoubleRow rhs of an fp8 layer.
            if 0 in fp8set:
                if xs8t_loaded is not None:
                    h3 = xs8t_loaded[:].rearrange("p (t b) -> p t b", t=3)
                else:
                    xs8t = p_xs.tile([128, 3 * GROUP], f8, tag="xs8")
                    for j in range(3):
                        if CP_ENG[j] == "A":
                            nc.scalar.activation(
                                xs8t[:, j * GROUP:(j + 1) * GROUP],
                                xs[j][:], Copy)
                        else:
                            nc.vector.tensor_copy(
                                out=xs8t[:, j * GROUP:(j + 1) * GROUP],
                                in_=xs[j][:])
                    h3 = xs8t[:].rearrange("p (t b) -> p t b", t=3)
            else:
                h3 = None

            bigps = cfg.get("bigps", False)

            def new_h(layer):
                """Output container for a layer: (slices, [p,t,b] view or
                None, full [128,1536] AP or None).  Single tile when the next
                layer consumes it via DoubleRow or when bigps needs span
                writes."""
                if (layer + 1) in fp8set:
                    ht3 = p_h.tile([128, 3 * GROUP], f8, tag=f"hp{layer}")
                    outs = [ht3[:, i * GROUP:(i + 1) * GROUP]
                            for i in range(3)]
                    return outs, ht3[:].rearrange("p (t b) -> p t b", t=3), \
                        ht3[:]
                if bigps:
                    ht3 = p_h.tile([128, 3 * GROUP], adt, tag=f"hb{layer}")
                    outs = [ht3[:, i * GROUP:(i + 1) * GROUP]
                            for i in range(3)]
                    return outs, None, ht3[:]
                outs = []
                for i in range(3):
                    hti = p_h.tile([128, GROUP], adt, tag=f"h{i}")
                    outs.append(hti[:])
                return outs, None, None

            def relu_layer(pst, out3, layer):
                """Whole-layer bias+relu, split across ACT and DVE."""
                if "tinyrelu" in probe:
                    nc.scalar.activation(out3[:, :16], pst[:, :16], Relu,
                                         bias=bt[layer])
                    return
                cut = cfg.get("rcut", 768)
                spans = ([("A", 0, cut), ("V", cut, 3 * GROUP)]
                         if layer % 2 == 0
                         else [("V", 0, cut), ("A", cut, 3 * GROUP)])
                for e, lo, hi in spans:
                    if e == "A":
                        nc.scalar.activation(out3[:, lo:hi], pst[:, lo:hi],
                                             Relu, bias=bt[layer])
                    else:
                        nc.vector.tensor_scalar(
                            out=out3[:, lo:hi], in0=pst[:, lo:hi],
                            scalar1=bt[layer], scalar2=0.0,
                            op0=mybir.AluOpType.add, op1=mybir.AluOpType.max)

            for layer in range(4):
                houts, h3_next, hfull = new_h(layer)
                pst = None
                if bigps:
                    pst = p_ph.tile([128, 3 * GROUP], f32, tag="ph")

                def psum_for(i):
                    if bigps:
                        return pst[:, i * GROUP:(i + 1) * GROUP]
                    psi = p_ph.tile([128, GROUP], f32, tag="ph")
                    return psi[:]

                hn = [None, None, None]
                if layer in fp8set:
                    assert h3 is not None
                    for i in iperm:
                        ps = psum_for(i)
                        plist = pair_by_li[(layer, i)]
                        for bi, (pi, j0) in enumerate(plist):
                            nc.tensor.matmul(
                                ps[:],
                                lhsT=wp8[:, pi * 256:(pi + 1) * 256]
                                    .rearrange("p (t m) -> p t m", t=2),
                                rhs=h3[:, j0:j0 + 2, :],
                                start=(bi == 0),
                                stop=(bi == len(plist) - 1),
                                perf_mode=DR)
                        if not bigps:
                            relu_move(RELU_ENG[layer][i], houts[i], ps, layer)
                        hn[i] = houts[i]
                    if bigps:
                        relu_layer(pst[:], hfull, layer)
                    h, h3 = hn, h3_next
                    continue
                if cfg.get("pack") and layer in (1, 3):
                    assert not bigps
                    # Layers with in-stacking S_B have two K=64 blocks (only
                    # one node of in-tile 0 feeds them).  Run them as two
                    # concurrent 64x128 row tiles, then the full blocks.
                    ko = woff[layer * 3]
                    ps0 = p_ph.tile([128, GROUP], f32, tag="ph")
                    ps1 = p_ph.tile([128, GROUP], f32, tag="ph")
                    ps2 = p_ph.tile([128, GROUP], f32, tag="ph")
                    kk = lambda i, bi: woff[layer * 3 + i] + bi
                    w_ = lambda k, lo, hi: wtile[lo:hi,
                                                 k * 128:(k + 1) * 128]
                    nc.tensor.matmul(ps0[:], lhsT=w_(kk(0, 0), 0, 64),
                                     rhs=h[0][0:64, :], start=True,
                                     stop=False, tile_position=(0, 0))
                    nc.tensor.matmul(ps2[:], lhsT=w_(kk(2, 0), 64, 128),
                                     rhs=h[0][64:128, :], start=True,
                                     stop=False, tile_position=(64, 0))
                    nc.tensor.matmul(ps0[:], lhsT=wt[kk(0, 1)], rhs=h[1][:],
                                     start=False, stop=True)
                    nc.tensor.matmul(ps1[:], lhsT=wt[kk(1, 0)], rhs=h[1][:],
                                     start=True, stop=False)
                    nc.tensor.matmul(ps1[:], lhsT=wt[kk(1, 1)], rhs=h[2][:],
                                     start=False, stop=True)
                    nc.tensor.matmul(ps2[:], lhsT=wt[kk(2, 1)], rhs=h[2][:],
                                     start=False, stop=True)
                    for i, ps in ((0, ps0), (1, ps1), (2, ps2)):
                        relu_move(RELU_ENG[layer][i], houts[i], ps, layer)
                        hn[i] = houts[i]
                    h, h3 = hn, h3_next
                    continue
                for i in iperm:
                    k = woff[layer * 3 + i]
                    ps = p_ph.tile([128, GROUP], f32, tag="ph")
                    js = BLOCK_PLAN[layer][i]
                    for bi, j in enumerate(js):
                        nc.tensor.matmul(
                            ps[:],
                            lhsT=wt[k],
                            rhs=h[j][:],
                            start=(bi == 0),
                            stop=(bi == len(js) - 1),
                        )
                        k += 1
                    relu_move(RELU_ENG[layer][i], houts[i], ps, layer)
                    hn[i] = houts[i]
                h, h3 = hn, h3_next

            if "nofc" in probe:
                # Probe mode: drop the residual/fc/sigmoid tail; store h[0]
                # directly so the pipeline still drains to DRAM.
                sig = p_sig.tile([NN, GROUP], f32, tag="sig")
                nc.vector.tensor_copy(out=sig[:], in_=h[0][:NN, :])
                nc.sync.dma_start(y_ap[g], sig[:])
                return first_t, None

            # Residual + fc heads: logits[n, b] accumulate in PSUM [6, 512]
            # with the tiny fc weights stationary (cheap weight loads, full
            # N=512 streams), then sigmoid (+bias) and a strided store
            # straight to the batch-major DRAM layout.
            psfc = p_pot.tile([NN, GROUP], f32, tag="ot")
            first_bm = None
            if cfg.get("fcres"):
                # Fold the residual into the fc head: logits = fc^T h + fc^T x
                # (6 cheap matmuls, no elementwise adds).
                for i in range(3):
                    mi = nc.tensor.matmul(psfc[:], lhsT=fct[i], rhs=h[i][:],
                                          start=(i == 0), stop=False)
                    if first_bm is None:
                        first_bm = mi
                    nc.tensor.matmul(psfc[:], lhsT=fct[i], rhs=xs[i][:],
                                     start=False, stop=(i == 2))
            else:
                for i in range(3):
                    ri = p_r.tile([128, GROUP], adt, tag=f"r{i}")
                    eng = nc.vector if ADD_ENG[i] == "V" else nc.gpsimd
                    eng.tensor_add(out=ri[:], in0=h[i][:], in1=xs[i][:])
                    mi = nc.tensor.matmul(
                        psfc[:],
                        lhsT=fct[i],
                        rhs=ri[:],
                        start=(i == 0),
                        stop=(i == 2),
                    )
                    if first_bm is None:
                        first_bm = mi
            sig = p_sig.tile([NN, GROUP], f32, tag="sig")
            nc.scalar.activation(sig[:], psfc[:], Sigmoid, bias=fcbt[:])
            # Store node-major [6, 512] contiguously; the host un-permutes.
            nc.sync.dma_start(y_ap[g], sig[:])
            return first_t, first_bm

        from concourse.tile_rust import add_dep_helper

        def run_groups():
            prev_bm = None
            for g in range(N_GROUPS):
                first_t, first_bm = group_body(g)
                if (prev_bm is not None and cfg.get("cluster", False)
                        and first_t is not None):
                    add_dep_helper(first_t.ins, prev_bm.ins, sync=False,
                                   reason="cluster transpose-mode runs")
                prev_bm = first_bm

        if repeats == 1:
            run_groups()
        else:
            hint = (mybir.EngineType.PE, mybir.EngineType.Activation,
                    mybir.EngineType.DVE, mybir.EngineType.SP,
                    mybir.EngineType.Pool)
            with tc.For_i(0, repeats, hint_engines=hint,
                          staggered_reset=cfg.get("stag", False)):
                run_groups()

    nc.compile()
    return nc


def build_program_v2(repeats: int = 1, cfg: dict | None = None):
    """Streamlined all-fp8 build: host-pretransposed x (bf16 + fp8), the
    _exec_plan matmul schedule (12 DoubleRow + 2 plain fp8 matmuls per
    group), per-op relu engine table, sigmoid/output-DMA batched over
    `sigb` groups, optional 2-group matmul interleave (`gpair`) to reuse
    stationary weights back-to-back."""
    import concourse.tile as tile
    import concourse.mybir as mybir
    from concourse import bacc

    cfg = dict(cfg or {})
    wave = cfg.get("wave", False)
    G = cfg.get("G", GROUP)           # batch elements per group
    NSUB = G // GROUP                 # 512-wide matmul sub-slices per group
    NG = PER_CORE // G
    gdef = 2 if G == GROUP else 1     # matmul-interleave width default
    bufs_xs = cfg.get("xs", (13 if G == GROUP else 7) if wave else 3)
    bufs_h = cfg.get("h", 4 if wave else 3)
    bufs_r = cfg.get("r", 3 if wave else 2)
    sigb = cfg.get("sigb", 2 if G == GROUP else 1)
    gsub = cfg.get("gsub", gdef if cfg.get("gpair") or wave else 1)
    fcres = cfg.get("fcres", False)
    _blk = max(sigb, gsub)
    bufs_pot = cfg.get("pot", 1)
    bufs_ph = cfg.get("ph", (8 - bufs_pot * sigb * NSUB) // NSUB)
    rtab = cfg.get("rtab", "AAV AVA VAV AVA").split()
    addeng = cfg.get("addeng", "V")

    f32 = mybir.dt.float32
    bf16 = mybir.dt.bfloat16
    f8 = mybir.dt.float8e4
    Relu = mybir.ActivationFunctionType.Relu
    Sigmoid = mybir.ActivationFunctionType.Sigmoid
    DR = mybir.MatmulPerfMode.DoubleRow
    NP = len(EXEC_PAIRS)

    nc = bacc.Bacc("TRN2", target_bir_lowering=False, debug=False,
                   num_devices=N_CORES)

    xt_ap = nc.dram_tensor("xt", [NG, 128, 3 * G], bf16,
                           kind="ExternalInput").ap()
    xt8_ap = nc.dram_tensor("xt8", [NG, 128, 3 * G], f8,
                            kind="ExternalInput").ap()
    y_ap = nc.dram_tensor("y", [NG, NN, G], f32,
                          kind="ExternalOutput").ap()
    wp8_ap = nc.dram_tensor("wpair8", [NP, 128, 256], f8,
                            kind="ExternalInput").ap()
    ws8_ap = nc.dram_tensor("wblk8", [N_BLOCKS, 128, 128], f8,
                            kind="ExternalInput").ap()
    bias_ap = nc.dram_tensor("bias", [4, 128], f32,
                             kind="ExternalInput").ap()
    fcw_ap = nc.dram_tensor("fcw", [3, 128, NN], bf16,
                            kind="ExternalInput").ap()
    fcb_ap = nc.dram_tensor("fcb", [NN, 1], f32, kind="ExternalInput").ap()

    blk = _blk
    assert NG % blk == 0 and blk % gsub == 0

    with tile.TileContext(nc) as tc, ExitStack() as ctx:
        cpool = ctx.enter_context(tc.tile_pool(name="consts", bufs=1))
        p_xs = ctx.enter_context(tc.tile_pool(name="xs", bufs=bufs_xs))
        p_h = ctx.enter_context(tc.tile_pool(name="h", bufs=bufs_h))
        p_r = ctx.enter_context(tc.tile_pool(name="r", bufs=bufs_r))
        p_sig = ctx.enter_context(tc.tile_pool(name="sig", bufs=2))
        p_ph = ctx.enter_context(tc.tile_pool(name="ph", bufs=bufs_ph,
                                              space="PSUM"))
        p_pot = ctx.enter_context(tc.tile_pool(name="pot", bufs=bufs_pot,
                                               space="PSUM"))

        btile = cpool.tile([128, 4], f32, tag="bias")
        nc.sync.dma_start(btile[:], bias_ap.rearrange("l p -> p l"))
        bt = [btile[:, layer:layer + 1] for layer in range(4)]
        ftile = cpool.tile([128, 3 * NN], bf16, tag="fcw")
        nc.sync.dma_start(ftile[:].rearrange("p (i n) -> p i n", i=3),
                          fcw_ap.rearrange("i p n -> p i n"))
        fct = [ftile[:, i * NN:(i + 1) * NN] for i in range(3)]
        fcbt = cpool.tile([NN, 1], f32, tag="fcb")
        nc.sync.dma_start(fcbt[:], fcb_ap[:])

        def load_x(g):
            x8 = p_xs.tile([128, 3 * G], f8, tag="xs8")
            xst = p_xs.tile([128, 3 * G], bf16, tag="xs")
            if "tinyx" in cfg.get("probe", ""):
                # Timing probe: per-slice 16-col DMAs keep the dependency
                # edges but ~zero the DMA volume (wrong numerics).
                for j in range(3):
                    o = j * G
                    nc.sync.dma_start(x8[:, o:o + 16], xt8_ap[g][:, o:o + 16])
                    nc.sync.dma_start(xst[:, o:o + 16], xt_ap[g][:, o:o + 16])
            else:
                nc.sync.dma_start(x8[:], xt8_ap[g])
                nc.sync.dma_start(xst[:], xt_ap[g])
            return xst, x8

        xb_pre = ({g: load_x(g) for g in range(min(2, NG))}
                  if repeats == 1 and not cfg.get("latext") else {})

        wp8 = cpool.tile([128, NP * 256], f8, tag="wpair8")
        nc.sync.dma_start(wp8[:].rearrange("p (k f) -> p k f", k=NP),
                          wp8_ap.rearrange("k p f -> p k f"))
        ws8 = cpool.tile([128, N_BLOCKS * 128], f8, tag="wblk8")
        nc.sync.dma_start(ws8[:].rearrange("p (k f) -> p k f", k=N_BLOCKS),
                          ws8_ap.rearrange("k p f -> p k f"))

        nodr = cfg.get("nodr", False)

        def emit_mm(ps, op, h3, start, stop, b0):
            """One matmul sub-slice: ps is the [128, 512] PSUM target AP,
            b0 the batch-column offset into the [128, t, G] h3 view."""
            bsl = slice(b0, b0 + GROUP)
            kind = op[0]
            if kind == "dr" and nodr:
                _, j0, step, pi = op
                ka, kb = EXEC_PAIRS[pi]
                nc.tensor.matmul(ps, lhsT=ws8[:, ka * 128:(ka + 1) * 128],
                                 rhs=h3[:, j0, bsl], start=start, stop=False)
                nc.tensor.matmul(ps, lhsT=ws8[:, kb * 128:(kb + 1) * 128],
                                 rhs=h3[:, j0 + step, bsl], start=False,
                                 stop=stop)
                return
            if kind == "dr":
                _, j0, step, pi = op
                rhs = (h3[:, j0:j0 + 2 * step - 1:step, bsl] if step == 2
                       else h3[:, j0:j0 + 2, bsl])
                nc.tensor.matmul(
                    ps,
                    lhsT=wp8[:, pi * 256:(pi + 1) * 256]
                        .rearrange("p (t m) -> p t m", t=2),
                    rhs=rhs, start=start, stop=stop, perf_mode=DR)
            else:
                _, j, k = op
                nc.tensor.matmul(
                    ps, lhsT=ws8[:, k * 128:(k + 1) * 128],
                    rhs=h3[:, j, bsl], start=start, stop=stop)

        probe = cfg.get("probe", "")

        def relu_move(e, out, ps, layer):
            if "tinyrelu" in probe:
                # Timing probe: keep dataflow edges, shrink the work (wrong
                # numerics) to expose the PE+DMA+schedule envelope.
                nc.scalar.activation(out[:, :16], ps[:, :16], Relu,
                                     bias=bt[layer])
                return
            if e == "A":
                nc.scalar.activation(out, ps[:], Relu, bias=bt[layer])
            else:
                nc.vector.tensor_scalar(out=out, in0=ps[:],
                                        scalar1=bt[layer], scalar2=0.0,
                                        op0=mybir.AluOpType.add,
                                        op1=mybir.AluOpType.max)

        latext = cfg.get("latext", False)

        def mk_st(g):
            if latext:
                x8 = p_xs.tile([128, 3 * G], f8, tag="xs8", name="x8")
                nc.sync.dma_start(x8[:], xt8_ap[g])
                return {"g": g, "xs": None,
                        "h3": x8[:].rearrange("p (t b) -> p t b", t=3),
                        "h": None}
            pre = xb_pre.pop(g, None)
            xst, x8 = pre if pre is not None else load_x(g)
            return {
                "g": g,
                "xs": [xst[:, j * G:(j + 1) * G] for j in range(3)],
                "h3": x8[:].rearrange("p (t b) -> p t b", t=3),
                "h": None,
            }

        def load_xt_into(st):
            # latext: the bf16 residual copy of x is only needed by the tail
            # stage, so load it one wave earlier instead of at pipeline entry.
            xst = p_xs.tile([128, 3 * G], bf16, tag="xs", name="xst",
                            bufs=cfg.get("xtb", 3))
            nc.sync.dma_start(xst[:], xt_ap[st["g"]])
            st["xs"] = [xst[:, j * G:(j + 1) * G] for j in range(3)]

        h3b = cfg.get("h3b", 7 if wave else bufs_h)

        def layer_step(ssts, layer):
            """One GCN layer for the groups in ssts, matmuls interleaved."""
            for st in ssts:
                if layer < 3:
                    ht3 = p_h.tile([128, 3 * G], f8, tag=f"hp{layer}")
                    st["houts"] = [ht3[:, i * G:(i + 1) * G]
                                   for i in range(3)]
                    st["h3n"] = ht3[:].rearrange("p (t b) -> p t b", t=3)
                else:
                    st["houts"] = [
                        p_h.tile([128, G], bf16, tag=f"h{i}",
                                 name=f"h{i}", bufs=h3b)[:]
                        for i in range(3)]
                    st["h3n"] = None
            for i in range(3):
                pss = [p_ph.tile([128, G], f32, tag="ph", name="ps")
                       for _ in ssts]
                ops = EXEC_OPS[(layer, i)]
                for hb in range(NSUB):
                    b0 = hb * GROUP
                    for bi, op in enumerate(ops):
                        for st, ps in zip(ssts, pss):
                            emit_mm(ps[:, b0:b0 + GROUP], op, st["h3"],
                                    start=(bi == 0),
                                    stop=(bi == len(ops) - 1), b0=b0)
                for st, ps in zip(ssts, pss):
                    relu_move(rtab[layer][i], st["houts"][i], ps, layer)
            for st in ssts:
                st["h3"] = st["h3n"]
                st["h"] = st["houts"]

        def tail_step(g0, csts):
            """Residual + fc heads + sigmoid + store for len(csts) groups."""
            nsig = len(csts)
            psfc = p_pot.tile([NN, nsig * G], f32, tag="ot", name="psfc")
            halves = [(s, s * G + hb * GROUP) for s in range(nsig)
                      for hb in range(NSUB)]

            def fc_mm(s, o, i, rhs_full, start, stop):
                b0 = o - s * G
                nc.tensor.matmul(psfc[:, o:o + GROUP], lhsT=fct[i],
                                 rhs=rhs_full[:, b0:b0 + GROUP],
                                 start=start, stop=stop)

            if fcres:
                for i in range(3):
                    for s, o in halves:
                        fc_mm(s, o, i, csts[s]["h"][i], i == 0, False)
                    for s, o in halves:
                        fc_mm(s, o, i, csts[s]["xs"][i], False, i == 2)
            else:
                for s, st in enumerate(csts):
                    rs = []
                    for i in range(3):
                        ri = p_r.tile([128, G], bf16, tag=f"r{i}")
                        eng = nc.vector if addeng == "V" else nc.gpsimd
                        if "tinyadd" in probe:
                            eng.tensor_add(out=ri[:, :16],
                                           in0=st["h"][i][:, :16],
                                           in1=st["xs"][i][:, :16])
                        else:
                            eng.tensor_add(out=ri[:], in0=st["h"][i],
                                           in1=st["xs"][i])
                        rs.append(ri)
                    for hb in range(NSUB):
                        o = s * G + hb * GROUP
                        for i in range(3):
                            fc_mm(s, o, i, rs[i][:], i == 0, i == 2)
            sig = p_sig.tile([NN, nsig * G], f32, tag="sig")
            nc.scalar.activation(sig[:], psfc[:], Sigmoid, bias=fcbt[:])
            # Optional: y stores on the Pool engine's DMA queue (measured
            # slower than the shared SP queue — keep off).
            dq_store = nc.gpsimd if cfg.get("dmaq", False) else nc.sync
            dq_store.dma_start(
                y_ap[g0:g0 + nsig].rearrange("g n b -> n g b"),
                sig[:].rearrange("n (g b) -> n g b", g=nsig))

        def blk_body(g0):
            sts = [mk_st(g) for g in range(g0, g0 + blk)]
            for s0 in range(0, blk, gsub):
                for layer in range(4):
                    layer_step(sts[s0:s0 + gsub], layer)
            for c0 in range(0, blk, sigb):
                tail_step(g0 + c0, sts[c0:c0 + sigb])

        def run_groups():
            if not cfg.get("wave"):
                for g0 in range(0, NG, blk):
                    blk_body(g0)
                return
            # Software-pipelined wavefront over pairs of groups: in wave w,
            # pair w loads its x, pair w-1 runs layer 0, ... pair w-4 runs
            # layer 3, pair w-5 runs the tail.  Every instruction in a wave
            # depends only on results from previous waves, so each engine's
            # strict-FIFO queue always has ready work at its head.
            W = gsub
            P = NG // W
            states = {}
            worder = cfg.get("worder", False)
            for w in range(P + 6):
                if worder and w < P:
                    states[w] = [mk_st(w * W + k) for k in range(W)]
                p = w - 5
                if 0 <= p < P:
                    tail_step(p * W, states.pop(p))
                for layer in (3, 2, 1, 0):
                    p = w - 1 - layer
                    if 0 <= p < P:
                        layer_step(states[p], layer)
                if latext:
                    p = w - 4
                    if 0 <= p < P:
                        for st in states[p]:
                            load_xt_into(st)
                if not worder and w < P:
                    states[w] = [mk_st(w * W + k) for k in range(W)]

        if repeats == 1:
            run_groups()
        else:
            hint = (mybir.EngineType.PE, mybir.EngineType.Activation,
                    mybir.EngineType.DVE, mybir.EngineType.SP,
                    mybir.EngineType.Pool)
            with tc.For_i(0, repeats, hint_engines=hint,
                          staggered_reset=cfg.get("stag", False)):
                run_groups()

    nc.compile()
    return nc


class Runner:
    """Compiled program + cached jitted PJRT executable over the 8 cores.

    Mirrors concourse.bass2jax.run_bass_via_pjrt, but keeps the jitted
    callable and accepts device-resident inputs so repeated timed calls do
    not re-trace or re-transfer."""

    def __init__(self, nc):
        import jax
        import numpy as _np
        from jax.sharding import Mesh, PartitionSpec, NamedSharding
        from jax.experimental.shard_map import shard_map
        import concourse.mybir as mybir
        from concourse import bass2jax

        bass2jax.install_neuronx_cc_hook()
        self.nc = nc
        assert nc.dbg_addr is None
        partition_name = (nc.partition_id_tensor.name
                          if nc.partition_id_tensor else None)

        in_names, out_names, out_avals, zero_outs = [], [], [], []
        for alloc in nc.m.functions[0].allocations:
            if not isinstance(alloc, mybir.MemoryLocationSet):
                continue
            name = alloc.memorylocations[0].name
            if alloc.kind == "ExternalInput":
                if name == partition_name:
                    continue
                in_names.append(name)
            elif alloc.kind == "ExternalOutput":
                shape = tuple(alloc.tensor_shape)
                dtype = mybir.dt.np(alloc.dtype)
                out_names.append(name)
                out_avals.append(jax.core.ShapedArray(shape, dtype))
                zero_outs.append(_np.zeros(shape, dtype))
        self.in_names = list(in_names)
        self.out_names = out_names
        self.out_avals = out_avals
        self.zero_outs = zero_outs
        n_params = len(in_names)
        n_outs = len(out_avals)
        all_in_names = in_names + out_names
        if partition_name is not None:
            all_in_names = all_in_names + [partition_name]

        def _body(*args):
            operands = list(args)
            if partition_name is not None:
                operands.append(bass2jax.partition_id_tensor())
            outs = bass2jax._bass_exec_p.bind(
                *operands,
                out_avals=tuple(out_avals),
                in_names=tuple(all_in_names),
                out_names=tuple(out_names),
                lowering_input_output_aliases=(),
                sim_require_finite=True,
                sim_require_nnan=True,
                nc=nc,
            )
            return tuple(outs)

        devices = jax.devices()[:N_CORES]
        self.mesh = Mesh(_np.asarray(devices), ("core",))
        self.sharding = NamedSharding(self.mesh, PartitionSpec("core"))
        in_specs = (PartitionSpec("core"),) * (n_params + n_outs)
        out_specs = (PartitionSpec("core"),) * n_outs
        self.jitted = jax.jit(
            shard_map(_body, mesh=self.mesh, in_specs=in_specs,
                      out_specs=out_specs, check_rep=False),
            keep_unused=True,
        )
        self._jax = jax

    def put_inputs(self, in_maps):
        """in_maps: list of N_CORES dicts name->np.  Returns device arrays."""
        import numpy as _np
        concat = [
            _np.concatenate([_np.asarray(m[name]) for m in in_maps], axis=0)
            for name in self.in_names
        ]
        dev = [self._jax.device_put(a, self.sharding) for a in concat]
        # The zero "output operands" are never read by the NEFF (no
        # input/output aliasing is declared); upload them once and reuse.
        self._zeros_dev = [
            self._jax.device_put(
                self._jax.numpy.zeros((N_CORES * z.shape[0], *z.shape[1:]),
                                      z.dtype),
                self.sharding)
            for z in self.zero_outs
        ]
        return dev

    def run(self, dev_inputs):
        outs = self.jitted(*dev_inputs, *self._zeros_dev)
        outs = [self._jax.block_until_ready(o) for o in outs]
        return {
            name: outs[i]
            for i, name in enumerate(self.out_names)
        }


_RUNNERS = {}


def get_runner(repeats: int = 1, cfg: dict | None = None) -> Runner:
    key = (repeats, tuple(sorted((cfg or {}).items())))
    if key not in _RUNNERS:
        build = build_program_v2 if (cfg or {}).get("v2") else build_program
        _RUNNERS[key] = Runner(build(repeats, cfg))
    return _RUNNERS[key]


def _make_in_maps(inputs, wdtype="f32r", xdtype=None):
    if xdtype is None:
        xdtype = wdtype if wdtype != "f32r" else "f32r"
    x = np.ascontiguousarray(np.asarray(inputs["x"], np.float32))
    assert x.shape == (BATCH, NN, FEAT)
    consts = build_consts(
        W=[np.asarray(inputs[f"W{i+1}"], np.float32) for i in range(4)],
        b=[np.asarray(inputs[f"b{i+1}"], np.float32) for i in range(4)],
        fc_w=np.asarray(inputs["fc_w"], np.float32),
        fc_b=np.asarray(inputs["fc_b"], np.float32),
    )
    if wdtype == "bf16":
        import ml_dtypes
        consts["wblk"] = consts["wblk"].astype(ml_dtypes.bfloat16)
        consts["fcw"] = consts["fcw"].astype(ml_dtypes.bfloat16)
    if xdtype == "bf16":
        import ml_dtypes
        x = x.astype(ml_dtypes.bfloat16)
        consts["eye128"] = consts["eye128"].astype(ml_dtypes.bfloat16)
    x_sh = x.reshape(N_CORES, PER_CORE, NN * FEAT)
    return [{"x": x_sh[c], **consts} for c in range(N_CORES)]


FP8_SETS = {None: frozenset(), "l234": frozenset({1, 2, 3}),
            "all": frozenset({0, 1, 2, 3})}


def _maps_for_v2(inputs, G=GROUP):
    """Input maps for build_program_v2: host-pretransposed x in bf16 + fp8,
    _exec_plan weight tensors, fc/bias consts."""
    import ml_dtypes
    NG = PER_CORE // G
    consts = build_consts(
        W=[np.asarray(inputs[f"W{i+1}"], np.float32) for i in range(4)],
        b=[np.asarray(inputs[f"b{i+1}"], np.float32) for i in range(4)],
        fc_w=np.asarray(inputs["fc_w"], np.float32),
        fc_b=np.asarray(inputs["fc_b"], np.float32),
    )
    wpair = np.zeros((len(EXEC_PAIRS), 128, 256), np.float32)
    for pi, (ka, kb) in enumerate(EXEC_PAIRS):
        wpair[pi, :, :128] = consts["wblk"][ka]
        wpair[pi, :, 128:] = consts["wblk"][kb]
    f8 = ml_dtypes.float8_e4m3
    com = {
        "wpair8": wpair.astype(f8),
        "wblk8": consts["wblk"].astype(f8),
        "bias": consts["bias"],
        "fcw": consts["fcw"].astype(ml_dtypes.bfloat16),
        "fcb": consts["fcb"],
    }
    x = np.ascontiguousarray(np.asarray(inputs["x"], np.float32))
    xt = np.ascontiguousarray(
        x.reshape(N_CORES, NG, G, 3, 128)
        .transpose(0, 1, 4, 3, 2)).reshape(N_CORES, NG, 128, 3 * G)
    xt_bf = xt.astype(ml_dtypes.bfloat16)
    xt_f8 = xt.astype(f8)
    return [{"xt": xt_bf[c], "xt8": xt_f8[c], **com} for c in range(N_CORES)]


def _maps_for_cfg(inputs, cfg):
    cfg = dict(cfg or {})
    if cfg.get("v2"):
        return _maps_for_v2(inputs, G=cfg.get("G", GROUP))
    xdt = cfg.get("xdtype", "f32r")
    wdt = cfg.get("wdtype", xdt)
    maps = _make_in_maps(inputs, wdtype=wdt, xdtype=xdt)
    fp8set = FP8_SETS[cfg.get("fp8")]
    if fp8set:
        import ml_dtypes
        consts = build_consts(
            W=[np.asarray(inputs[f"W{i+1}"], np.float32) for i in range(4)],
            b=[np.asarray(inputs[f"b{i+1}"], np.float32) for i in range(4)],
            fc_w=np.asarray(inputs["fc_w"], np.float32),
            fc_b=np.asarray(inputs["fc_b"], np.float32),
        )
        pairs = _pair_plan(fp8set)
        wpair = np.zeros((len(pairs), 128, 256), np.float32)
        for pi, (_l, _i, _j0, (ka, kb)) in enumerate(pairs):
            if ka is not None:
                wpair[pi, :, :128] = consts["wblk"][ka]
            if kb is not None:
                wpair[pi, :, 128:] = consts["wblk"][kb]
        wp8 = wpair.astype(ml_dtypes.float8_e4m3)
        for m in maps:
            m["wpair8"] = wp8
    if cfg.get("hostt"):
        import ml_dtypes
        x = np.ascontiguousarray(np.asarray(inputs["x"], np.float32))
        # [C, NG, 512, 3, 128] -> feature-major [C, NG, 128(p), 3(j), 512(b)]
        xt = np.ascontiguousarray(
            x.reshape(N_CORES, N_GROUPS, GROUP, 3, 128)
            .transpose(0, 1, 4, 3, 2)).reshape(N_CORES, N_GROUPS, 128,
                                               3 * GROUP)
        xt_bf = xt.astype(ml_dtypes.bfloat16)
        xt_f8 = (xt.astype(ml_dtypes.float8_e4m3)
                 if 0 in fp8set else None)
        for c, m in enumerate(maps):
            m.pop("x", None)
            m["xt"] = xt_bf[c]
            if xt_f8 is not None:
                m["xt8"] = xt_f8[c]
    return maps


def unpack_y(y_raw: np.ndarray, G: int = GROUP) -> np.ndarray:
    """Device output [N_CORES * NG, NN, G] -> [BATCH, NN]."""
    NG = PER_CORE // G
    y = y_raw.reshape(N_CORES, NG, NN, G)
    return np.ascontiguousarray(
        y.transpose(0, 1, 3, 2).reshape(BATCH, NN))


# Best measured configuration (see NOTES.md): all-fp8 exec plan on 1024-wide
# groups, 6-stage software-pipelined wave schedule, bf16-x loaded at a late
# wave stage, staggered semaphore reset on the repeat loop.
DEFAULT_CFG = {"v2": True, "G": 1024, "wave": True, "latext": True,
               "stag": True}


def kernel(**inputs) -> np.ndarray:
    runner = get_runner(repeats=1, cfg=DEFAULT_CFG)
    dev = runner.put_inputs(_maps_for_cfg(inputs, DEFAULT_CFG))
    out = runner.run(dev)
    return unpack_y(np.asarray(out["y"]), G=DEFAULT_CFG.get("G", GROUP))

